# revision 4
# baseline (speedup 1.0000x reference)
"""Multi-head attention (B=4, S=2048, D=1024, H=16, causal) on 8 trn2 NeuronCores.

Sharding: core i handles batch b = i//2 and head-group hg = i%2 (8 heads each).
Data-parallel over B, tensor-parallel over heads; the out-projection partial
sums of the two head-groups of a batch are reduced on the host. No collectives.

Per-core dataflow (all matmuls in float32r, typed end-to-end — the BIR
verifier requires fp32r matmul operands to be produced as fp32r):
  phase A: QT[o,s], KT[o,s] (head-transposed) and V[s,o] (natural, augmented
           with a ones-column per head) via projections from host-transposed
           activations; biases folded in with K=1 augmented matmuls.
  phase B: per (q-tile, head): scores T[k,q] = KT_blk @ QT_tile on PE,
           exp on ScalarE (no max subtraction; causal scores are O(+-6)),
           causal masking via block skipping + one triangular mask tile,
           P.T @ [V|1] accumulation gives attention output (transposed) and
           softmax denominators in one PSUM tile; normalize via PE-broadcast
           of reciprocal denominators.
  phase C: out-projection with A.T blocks as stationary operands; per-core
           partial y (bo added on host).
"""

import numpy as np
from contextlib import ExitStack

import concourse.bass as bass
import concourse.tile as tile
from concourse import bacc, mybir
from concourse.bass_utils import run_bass_kernel_spmd

F32 = mybir.dt.float32
F32R = mybir.dt.float32r

B, S, D, H, DK = 4, 2048, 1024, 16, 64
HPC = 8          # heads per core
OC = HPC * DK    # 512 out-cols per core
NB = S // 128    # 16 seq blocks of 128
NQT = S // 512   # 4 q-tiles of 512
N_CORES = 8
VSTRIDE = DK + 1  # V cols per head incl the ones column


def build_program(cls, mask_tiles, causal):
    """cls[kb][qb] for the 16x16 grid of 128x128 blocks (T orientation:
    kb = key block, qb = query block): 0 = fully masked, 1 = fully valid,
    >=2 -> mixed, multiply by mask_tiles[cls-2] after exp."""
    n_mask = len(mask_tiles)
    nc = bacc.Bacc("TRN2", target_bir_lowering=False, debug=False,
                   num_devices=N_CORES, enable_asserts=False)

    xqT = nc.dram_tensor("xqT", [D, S], F32R, kind="ExternalInput").ap()
    xkT = nc.dram_tensor("xkT", [D, S], F32R, kind="ExternalInput").ap()
    xvT = nc.dram_tensor("xvT", [D, S], F32R, kind="ExternalInput").ap()
    wqT = nc.dram_tensor("wqT", [D, OC], F32R, kind="ExternalInput").ap()
    wkT = nc.dram_tensor("wkT", [D, OC], F32R, kind="ExternalInput").ap()
    wvT = nc.dram_tensor("wvT", [D, OC], F32R, kind="ExternalInput").ap()
    bq = nc.dram_tensor("bq", [OC], F32R, kind="ExternalInput").ap()
    bk = nc.dram_tensor("bk", [OC], F32R, kind="ExternalInput").ap()
    bv = nc.dram_tensor("bv", [OC], F32R, kind="ExternalInput").ap()
    woT = nc.dram_tensor("woT", [OC, D], F32R, kind="ExternalInput").ap()
    ones_row_d = nc.dram_tensor("ones_row", [512], F32R,
                                kind="ExternalInput").ap()
    ones_va_d = nc.dram_tensor("ones_va", [128, NB, HPC], F32R,
                               kind="ExternalInput").ap()
    zeros_d = None
    if not causal:
        zeros_d = nc.dram_tensor("zeros", [128, 512], F32R,
                                 kind="ExternalInput").ap()
    masks = None
    if n_mask:
        masks = nc.dram_tensor("masks", [n_mask, 128, 128], F32R,
                               kind="ExternalInput").ap()
    y = nc.dram_tensor("y", [S, D], F32, kind="ExternalOutput").ap()

    with tile.TileContext(nc) as tc, ExitStack() as ctx:
        persist = ctx.enter_context(tc.tile_pool(name="persist", bufs=1))
        QT = persist.tile([128, 4, S], F32R, tag="QT")
        KT = persist.tile([128, 4, S], F32R, tag="KT")
        VA = persist.tile([128, NB, HPC * VSTRIDE], F32R, tag="VA")
        ones = persist.tile([1, 512], F32R, tag="ones")
        nc.sync.dma_start(out=ones, in_=ones_row_d[None, :])
        zeros = None
        if zeros_d is not None:
            zeros = persist.tile([128, 512], F32R, tag="zeros")
            nc.sync.dma_start(out=zeros, in_=zeros_d)
        bq_sb = persist.tile([1, OC], F32R, tag="bq")
        bk_sb = persist.tile([1, OC], F32R, tag="bk")
        bv_sb = persist.tile([1, OC], F32R, tag="bv")
        nc.sync.dma_start(out=bq_sb, in_=bq[None, :])
        nc.sync.dma_start(out=bk_sb, in_=bk[None, :])
        nc.sync.dma_start(out=bv_sb, in_=bv[None, :])
        mask_sb = None
        if n_mask:
            mask_sb = persist.tile([128, n_mask, 128], F32R, tag="mask")
            nc.sync.dma_start(out=mask_sb, in_=masks.rearrange("m p j -> p m j"))
        # ones column of the augmented V
        nc.sync.dma_start(
            out=VA.rearrange("p b (h e) -> p b h e", e=VSTRIDE)[:, :, :, DK],
            in_=ones_va_d)

        # ---------------- phase A: projections ----------------
        with tc.tile_pool(name="phA_w", bufs=1) as wpool, \
             tc.tile_pool(name="phA_x", bufs=2) as xpool, \
             tc.tile_pool(name="phA_ps", bufs=4, space=bass.MemorySpace.PSUM) as pps:
            wq_sb = wpool.tile([128, 8, OC], F32R, tag="wq")
            wk_sb = wpool.tile([128, 8, OC], F32R, tag="wk")
            wv_sb = wpool.tile([128, 8, OC], F32R, tag="wv")
            nc.sync.dma_start(out=wq_sb, in_=wqT.rearrange("(c p) o -> p c o", p=128))
            nc.sync.dma_start(out=wk_sb, in_=wkT.rearrange("(c p) o -> p c o", p=128))
            nc.sync.dma_start(out=wv_sb, in_=wvT.rearrange("(c p) o -> p c o", p=128))

            for st in range(NQT):
                ssl = slice(512 * st, 512 * (st + 1))
                for which in ("q", "k", "v"):
                    xsrc = {"q": xqT, "k": xkT, "v": xvT}[which]
                    xs = xpool.tile([128, 8, 512], F32R, tag="xstage")
                    nc.sync.dma_start(
                        out=xs,
                        in_=xsrc.rearrange("(c p) s -> p c s", p=128)[:, :, ssl])
                    if which in ("q", "k"):
                        wsb = wq_sb if which == "q" else wk_sb
                        bsb = bq_sb if which == "q" else bk_sb
                        dst = QT if which == "q" else KT
                        for ob in range(4):
                            osl = slice(128 * ob, 128 * (ob + 1))
                            ps = pps.tile([128, 512], F32, tag="ps")
                            for c in range(8):
                                nc.tensor.matmul(ps, wsb[:, c, osl], xs[:, c, :],
                                                 start=(c == 0), stop=False)
                            nc.tensor.matmul(ps, bsb[:, osl], ones,
                                             start=False, stop=True)
                            if which == "q":
                                nc.vector.tensor_copy(dst[:, ob, ssl], ps)
                            else:
                                nc.scalar.copy(dst[:, ob, ssl], ps)
                    else:
                        for s2 in range(4):
                            sb = 4 * st + s2
                            ps = pps.tile([128, 512], F32, tag="ps")
                            for c in range(8):
                                nc.tensor.matmul(
                                    ps, xs[:, c, 128 * s2:128 * (s2 + 1)],
                                    wv_sb[:, c, :],
                                    start=(c == 0), stop=False)
                            nc.tensor.matmul(ps, ones[:, 0:128], bv_sb,
                                             start=False, stop=True)
                            nc.vector.tensor_copy(
                                VA[:, sb, :].rearrange("p (h e) -> p h e",
                                                       e=VSTRIDE)[:, :, 0:DK],
                                ps.rearrange("p (h e) -> p h e", e=DK))

        # ---------------- phases B + C ----------------
        with tc.tile_pool(name="phB", bufs=1) as bpool, \
             tc.tile_pool(name="pt", bufs=3) as ptpool, \
             tc.tile_pool(name="bcs", bufs=2) as bcpool, \
             tc.tile_pool(name="rc", bufs=2) as rcpool, \
             tc.tile_pool(name="outst", bufs=3) as opool, \
             tc.tile_pool(name="psT", bufs=2, space=bass.MemorySpace.PSUM) as psT, \
             tc.tile_pool(name="psAV", bufs=2, space=bass.MemorySpace.PSUM) as psAV, \
             tc.tile_pool(name="psBC", bufs=2, space=bass.MemorySpace.PSUM) as psBC, \
             tc.tile_pool(name="psO", bufs=2, space=bass.MemorySpace.PSUM) as psO:
            AT = bpool.tile([128, 4, S], F32R, tag="AT")
            wo_sb = bpool.tile([128, 4, D], F32R, tag="wo")
            nc.sync.dma_start(out=wo_sb, in_=woT.rearrange("(c p) n -> p c n", p=128))

            for qt in range(NQT):
                qsl = slice(512 * qt, 512 * (qt + 1))
                for h in range(HPC):
                    ob, hf = h // 2, (h % 2) * DK
                    q_ap = QT[hf:hf + DK, ob, qsl]
                    vsl = slice(VSTRIDE * h, VSTRIDE * h + VSTRIDE)
                    active = [kb for kb in range(NB)
                              if any(cls[kb][4 * qt + j] for j in range(4))]
                    if not active:
                        nc.vector.tensor_copy(AT[hf:hf + DK, ob, qsl],
                                              zeros[0:DK, :])
                        continue
                    av = psAV.tile([DK + 1, 512], F32, tag="av")
                    pending = None  # (kb, ptile, c0) awaiting its AV matmul

                    def flush(stop):
                        kb_, pt_, c0_ = pending
                        nc.tensor.matmul(
                            av[:, 128 * c0_:], VA[:, kb_, vsl],
                            pt_[:, 128 * c0_:],
                            start=(kb_ == active[0]), stop=stop)

                    for kb in active:
                        sub = [cls[kb][4 * qt + j] for j in range(4)]
                        if causal:
                            c0 = kb - 4 * qt if kb >= 4 * qt else 0
                        else:
                            c0 = 0
                        if kb == active[0]:
                            c0 = 0  # first AV matmul must cover all columns
                        pt_ps = psT.tile([128, 512], F32, tag="pt")
                        nc.tensor.matmul(pt_ps[:, 128 * c0:],
                                         KT[hf:hf + DK, ob,
                                            128 * kb:128 * (kb + 1)],
                                         q_ap[:, 128 * c0:],
                                         start=True, stop=True)
                        ptile = ptpool.tile([128, 512], F32R, tag="ptile")
                        nc.scalar.activation(
                            ptile[:, 128 * c0:], pt_ps[:, 128 * c0:],
                            mybir.ActivationFunctionType.Exp, scale=0.125)
                        for j in range(c0, 4):
                            jsl = slice(128 * j, 128 * (j + 1))
                            if sub[j] == 0:
                                nc.vector.tensor_copy(ptile[:, jsl],
                                                      zeros[:, 0:128])
                            elif sub[j] >= 2:
                                nc.vector.tensor_mul(
                                    ptile[:, jsl], ptile[:, jsl],
                                    mask_sb[:, sub[j] - 2, :])
                        if pending is not None:
                            flush(stop=False)
                        pending = (kb, ptile, c0)
                    flush(stop=True)

                    rc = rcpool.tile([1, 512], F32, tag="rc")
                    nc.vector.reciprocal(rc, av[DK:DK + 1, :])
                    rcr = rcpool.tile([1, 512], F32R, tag="rcr")
                    nc.vector.tensor_copy(rcr, rc)
                    bc_ps = psBC.tile([DK, 512], F32, tag="bc")
                    nc.tensor.matmul(bc_ps, ones[:, 0:DK], rcr,
                                     start=True, stop=True)
                    bcs = bcpool.tile([DK, 512], F32, tag="bcs")
                    nc.scalar.copy(bcs, bc_ps)
                    nc.vector.tensor_mul(AT[hf:hf + DK, ob, qsl],
                                         av[0:DK, :], bcs)

                # out-projection for this q-tile
                for s2 in range(4):
                    sb = 4 * qt + s2
                    for ct in range(2):
                        csl = slice(512 * ct, 512 * (ct + 1))
                        po = psO.tile([128, 512], F32, tag="po")
                        for hb in range(4):
                            nc.tensor.matmul(
                                po, AT[:, hb, 128 * sb:128 * (sb + 1)],
                                wo_sb[:, hb, csl],
                                start=(hb == 0), stop=(hb == 3))
                        osb = opool.tile([128, 512], F32, tag="osb")
                        nc.vector.tensor_copy(osb, po)
                        nc.sync.dma_start(
                            out=y.rearrange("(b p) c -> b p c", p=128)[sb][:, csl],
                            in_=osb)

    nc.compile()
    return nc


def _classify_mask(mask2d):
    """Return (cls 16x16 list, mask_tiles list, causal flag) for the T
    orientation: cls[kb][qb] over 128x128 blocks of mask2d[q, k]."""
    m = (np.asarray(mask2d) != 0)
    blocks = m.reshape(NB, 128, NB, 128)  # [qb, ql, kb, kl]
    cls = [[0] * NB for _ in range(NB)]
    tiles = []
    keys = {}
    for kb in range(NB):
        for qb in range(NB):
            blk = blocks[qb, :, kb, :]  # [ql, kl]
            s = int(blk.sum())
            if s == 0:
                cls[kb][qb] = 0
            elif s == 128 * 128:
                cls[kb][qb] = 1
            else:
                t = np.ascontiguousarray(blk.T).astype(np.float32)  # [kl, ql]
                key = t.tobytes()
                if key not in keys:
                    keys[key] = len(tiles)
                    tiles.append(t)
                cls[kb][qb] = 2 + keys[key]
    causal = bool(np.array_equal(m, np.tril(np.ones((S, S), bool))))
    return cls, tiles, causal


_PROGRAM_CACHE = {}


def _get_program(mask2d):
    cls, tiles, causal = _classify_mask(mask2d)
    key = (tuple(tuple(r) for r in cls),
           tuple(t.tobytes() for t in tiles), causal)
    if key not in _PROGRAM_CACHE:
        _PROGRAM_CACHE[key] = (build_program(cls, tiles, causal), tiles, causal)
    return _PROGRAM_CACHE[key]


def run(inputs, trace=False):
    query = np.asarray(inputs["query"], np.float32)
    key_ = np.asarray(inputs["key"], np.float32)
    value = np.asarray(inputs["value"], np.float32)
    mask = np.asarray(inputs["mask"])
    Wq = np.asarray(inputs["Wq"], np.float32)
    bq = np.asarray(inputs["bq"], np.float32)
    Wk = np.asarray(inputs["Wk"], np.float32)
    bk = np.asarray(inputs["bk"], np.float32)
    Wv = np.asarray(inputs["Wv"], np.float32)
    bv = np.asarray(inputs["bv"], np.float32)
    Wo = np.asarray(inputs["Wo"], np.float32)
    bo = np.asarray(inputs["bo"], np.float32)

    nc, tiles, causal_flag = _get_program(mask[0, 0])

    in_maps = []
    for core in range(N_CORES):
        b, hg = core // 2, core % 2
        osl = slice(OC * hg, OC * (hg + 1))
        im = {
            "xqT": np.ascontiguousarray(query[b].T),
            "xkT": np.ascontiguousarray(key_[b].T),
            "xvT": np.ascontiguousarray(value[b].T),
            "wqT": np.ascontiguousarray(Wq.T[:, osl]),
            "wkT": np.ascontiguousarray(Wk.T[:, osl]),
            "wvT": np.ascontiguousarray(Wv.T[:, osl]),
            "bq": bq[osl].copy(),
            "bk": bk[osl].copy(),
            "bv": bv[osl].copy(),
            "woT": np.ascontiguousarray(Wo.T[osl, :]),
            "ones_row": np.ones(512, np.float32),
            "ones_va": np.ones((128, NB, HPC), np.float32),
        }
        if not causal_flag:
            im["zeros"] = np.zeros((128, 512), np.float32)
        if tiles:
            im["masks"] = np.stack(tiles)
        in_maps.append(im)

    res = run_bass_kernel_spmd(nc, in_maps, list(range(N_CORES)), trace=trace)
    out = np.empty((B, S, D), np.float32)
    for b in range(B):
        out[b] = res.results[2 * b]["y"] + res.results[2 * b + 1]["y"]
    out += bo
    return out, res


def kernel(**inputs):
    out, _ = run(inputs, trace=False)
    return out


# revision 8
# speedup vs baseline: 1.1827x; 1.1827x over previous
"""Multi-head attention (B=4, S=2048, D=1024, H=16, causal) on 8 trn2 NeuronCores.

Sharding: core i handles batch b = i//2 and head-group hg = i%2 (8 heads each).
Data-parallel over B, tensor-parallel over heads; the out-projection partial
sums of the two head-groups of a batch are reduced on the host. No collectives.

Per-core dataflow (all matmuls in float32r, typed end-to-end — the BIR
verifier requires fp32r matmul operands to be produced as fp32r):
  phase A: QT[o,s], KT[o,s] (head-transposed) and V[s,o] (natural, augmented
           with a ones-column per head) via projections from host-transposed
           activations; biases folded in with K=1 augmented matmuls.
  phase B: per (q-tile, head): scores T[k,q] = KT_blk @ QT_tile on PE,
           exp on ScalarE (no max subtraction; causal scores are O(+-6)),
           causal masking via block skipping + one triangular mask tile,
           P.T @ [V|1] accumulation gives attention output (transposed) and
           softmax denominators in one PSUM tile; normalize via PE-broadcast
           of reciprocal denominators.
  phase C: out-projection with A.T blocks as stationary operands; per-core
           partial y (bo added on host).
"""

import numpy as np
from contextlib import ExitStack

import concourse.bass as bass
import concourse.tile as tile
from concourse import bacc, mybir
from concourse.bass_utils import run_bass_kernel_spmd

F32 = mybir.dt.float32
F32R = mybir.dt.float32r

B, S, D, H, DK = 4, 2048, 1024, 16, 64
HPC = 8          # heads per core
OC = HPC * DK    # 512 out-cols per core
NB = S // 128    # 16 seq blocks of 128
NQT = S // 512   # 4 q-tiles of 512
N_CORES = 8
VSTRIDE = DK + 1  # V cols per head incl the ones column


def build_program(cls, mask_tiles, causal):
    """cls[kb][qb] for the 16x16 grid of 128x128 blocks (T orientation:
    kb = key block, qb = query block): 0 = fully masked, 1 = fully valid,
    >=2 -> mixed, multiply by mask_tiles[cls-2] after exp."""
    n_mask = len(mask_tiles)
    nc = bacc.Bacc("TRN2", target_bir_lowering=False, debug=False,
                   num_devices=N_CORES, enable_asserts=False)

    xqT = nc.dram_tensor("xqT", [D, S], F32R, kind="ExternalInput").ap()
    xkT = nc.dram_tensor("xkT", [D, S], F32R, kind="ExternalInput").ap()
    xvT = nc.dram_tensor("xvT", [D, S], F32R, kind="ExternalInput").ap()
    wqT = nc.dram_tensor("wqT", [D, OC], F32R, kind="ExternalInput").ap()
    wkT = nc.dram_tensor("wkT", [D, OC], F32R, kind="ExternalInput").ap()
    wvT = nc.dram_tensor("wvT", [D, OC], F32R, kind="ExternalInput").ap()
    bq = nc.dram_tensor("bq", [OC], F32R, kind="ExternalInput").ap()
    bk = nc.dram_tensor("bk", [OC], F32R, kind="ExternalInput").ap()
    bv = nc.dram_tensor("bv", [OC], F32R, kind="ExternalInput").ap()
    woT = nc.dram_tensor("woT", [OC, D], F32R, kind="ExternalInput").ap()
    ones_row_d = nc.dram_tensor("ones_row", [512], F32R,
                                kind="ExternalInput").ap()
    consts_va_d = nc.dram_tensor("consts_va", [128, NB, HPC + 64], F32R,
                                 kind="ExternalInput").ap()
    zeros_d = nc.dram_tensor("zeros", [128, 512], F32R,
                             kind="ExternalInput").ap()
    masks = None
    if n_mask:
        masks = nc.dram_tensor("masks", [n_mask, 128, 128], F32R,
                               kind="ExternalInput").ap()
    y = nc.dram_tensor("y", [S, D], F32, kind="ExternalOutput").ap()

    with tile.TileContext(nc) as tc, ExitStack() as ctx:
        persist = ctx.enter_context(tc.tile_pool(name="persist", bufs=1))
        QT = persist.tile([128, 4, S], F32R, tag="QT")
        KT = persist.tile([128, 4, S], F32R, tag="KT")
        # VSTRIDE*HPC data cols + 64 zero pad cols so the AV stationary can
        # always be a full [128, 128] window (M=128 is the fast LDW path)
        VA = persist.tile([128, NB, HPC * VSTRIDE + 64], F32R, tag="VA")
        ones = persist.tile([1, 512], F32R, tag="ones")
        nc.sync.dma_start(out=ones, in_=ones_row_d[None, :])
        zeros = None
        if not causal:
            zeros = persist.tile([128, 512], F32R, tag="zeros")
            nc.sync.dma_start(out=zeros, in_=zeros_d)
        # Q staging tiles, zero-padded so the scores matmul can contract over
        # the full 128 partitions (two-head KT block x one-head padded Q).
        # Even h uses qpad0 (head rows 0:64), odd h uses qpad1 (rows 64:128);
        # the other half of each stays zero forever.
        qpad0 = persist.tile([128, 512], F32R, tag="qpad0")
        qpad1 = persist.tile([128, 512], F32R, tag="qpad1")
        nc.sync.dma_start(out=qpad0, in_=zeros_d)
        nc.sync.dma_start(out=qpad1, in_=zeros_d)
        bq_sb = persist.tile([1, OC], F32R, tag="bq")
        bk_sb = persist.tile([1, OC], F32R, tag="bk")
        bv_sb = persist.tile([1, OC], F32R, tag="bv")
        nc.sync.dma_start(out=bq_sb, in_=bq[None, :])
        nc.sync.dma_start(out=bk_sb, in_=bk[None, :])
        nc.sync.dma_start(out=bv_sb, in_=bv[None, :])
        mask_sb = None
        if n_mask:
            mask_sb = persist.tile([128, n_mask, 128], F32R, tag="mask")
            nc.sync.dma_start(out=mask_sb, in_=masks.rearrange("m p j -> p m j"))
        # ones column of the augmented V + zero tail pad
        for h in range(HPC):
            nc.sync.dma_start(
                out=VA[:, :, VSTRIDE * h + DK:VSTRIDE * h + DK + 1],
                in_=consts_va_d[:, :, h:h + 1])
        nc.sync.dma_start(
            out=VA[:, :, HPC * VSTRIDE:HPC * VSTRIDE + 64],
            in_=consts_va_d[:, :, HPC:HPC + 64])

        # ---------------- phase A: projections ----------------
        with tc.tile_pool(name="phA_w", bufs=1) as wpool, \
             tc.tile_pool(name="phA_x", bufs=2) as xpool, \
             tc.tile_pool(name="phA_ps", bufs=4, space=bass.MemorySpace.PSUM) as pps:
            wq_sb = wpool.tile([128, 8, OC], F32R, tag="wq")
            wk_sb = wpool.tile([128, 8, OC], F32R, tag="wk")
            wv_sb = wpool.tile([128, 8, OC], F32R, tag="wv")
            nc.sync.dma_start(out=wq_sb, in_=wqT.rearrange("(c p) o -> p c o", p=128))

            for which in ("q", "k", "v"):
                if which == "k":
                    nc.sync.dma_start(
                        out=wk_sb, in_=wkT.rearrange("(c p) o -> p c o", p=128))
                if which == "v":
                    nc.sync.dma_start(
                        out=wv_sb, in_=wvT.rearrange("(c p) o -> p c o", p=128))
                for st in range(NQT):
                    ssl = slice(512 * st, 512 * (st + 1))
                    xsrc = {"q": xqT, "k": xkT, "v": xvT}[which]
                    xs = xpool.tile([128, 8, 512], F32R, tag="xstage")
                    nc.sync.dma_start(
                        out=xs,
                        in_=xsrc.rearrange("(c p) s -> p c s", p=128)[:, :, ssl])
                    if which in ("q", "k"):
                        wsb = wq_sb if which == "q" else wk_sb
                        bsb = bq_sb if which == "q" else bk_sb
                        dst = QT if which == "q" else KT
                        for ob in range(4):
                            osl = slice(128 * ob, 128 * (ob + 1))
                            ps = pps.tile([128, 512], F32, tag="ps")
                            for c in range(8):
                                nc.tensor.matmul(ps, wsb[:, c, osl], xs[:, c, :],
                                                 start=(c == 0), stop=False)
                            nc.tensor.matmul(ps, bsb[:, osl], ones,
                                             start=False, stop=True)
                            if which == "q":
                                nc.vector.tensor_copy(dst[:, ob, ssl], ps)
                            else:
                                nc.scalar.copy(dst[:, ob, ssl], ps)
                    else:
                        for s2 in range(4):
                            sb = 4 * st + s2
                            ps = pps.tile([128, 512], F32, tag="ps")
                            for c in range(8):
                                nc.tensor.matmul(
                                    ps, xs[:, c, 128 * s2:128 * (s2 + 1)],
                                    wv_sb[:, c, :],
                                    start=(c == 0), stop=False)
                            nc.tensor.matmul(ps, ones[:, 0:128], bv_sb,
                                             start=False, stop=True)
                            nc.vector.tensor_copy(
                                VA[:, sb, 0:HPC * VSTRIDE].rearrange(
                                    "p (h e) -> p h e",
                                    e=VSTRIDE)[:, :, 0:DK],
                                ps.rearrange("p (h e) -> p h e", e=DK))

        # ---------------- phases B + C ----------------
        with tc.tile_pool(name="phB", bufs=1) as bpool, \
             tc.tile_pool(name="pt", bufs=3) as ptpool, \
             tc.tile_pool(name="bcs", bufs=2) as bcpool, \
             tc.tile_pool(name="rc", bufs=2) as rcpool, \
             tc.tile_pool(name="outst", bufs=3) as opool, \
             tc.tile_pool(name="psT", bufs=2, space=bass.MemorySpace.PSUM) as psT, \
             tc.tile_pool(name="psAV", bufs=2, space=bass.MemorySpace.PSUM) as psAV, \
             tc.tile_pool(name="psBC", bufs=2, space=bass.MemorySpace.PSUM) as psBC, \
             tc.tile_pool(name="psO", bufs=2, space=bass.MemorySpace.PSUM) as psO:
            AT = bpool.tile([128, 4, S], F32R, tag="AT")
            wo_sb = bpool.tile([128, 4, D], F32R, tag="wo")
            nc.sync.dma_start(out=wo_sb, in_=woT.rearrange("(c p) n -> p c n", p=128))

            for qt in range(NQT):
                qsl = slice(512 * qt, 512 * (qt + 1))
                for h in range(HPC):
                    ob, hf = h // 2, (h % 2) * DK
                    qpad = qpad0 if h % 2 == 0 else qpad1
                    nc.vector.tensor_copy(qpad[hf:hf + DK, :],
                                          QT[hf:hf + DK, ob, qsl])
                    vsl = slice(VSTRIDE * h, VSTRIDE * h + 128)
                    active = [kb for kb in range(NB)
                              if any(cls[kb][4 * qt + j] for j in range(4))]
                    if not active:
                        nc.vector.tensor_copy(AT[hf:hf + DK, ob, qsl],
                                              zeros[0:DK, :])
                        continue
                    av = psAV.tile([128, 512], F32, tag="av")
                    pending = None  # (kb, ptile, c0) awaiting its AV matmul

                    def flush(stop):
                        kb_, pt_, c0_ = pending
                        nc.tensor.matmul(
                            av[:, 128 * c0_:], VA[:, kb_, vsl],
                            pt_[:, 128 * c0_:],
                            start=(kb_ == active[0]), stop=stop)

                    for kb in active:
                        sub = [cls[kb][4 * qt + j] for j in range(4)]
                        if causal:
                            c0 = kb - 4 * qt if kb >= 4 * qt else 0
                        else:
                            c0 = 0
                        if kb == active[0]:
                            c0 = 0  # first AV matmul must cover all columns
                        pt_ps = psT.tile([128, 512], F32, tag="pt")
                        nc.tensor.matmul(pt_ps[:, 128 * c0:],
                                         KT[:, ob, 128 * kb:128 * (kb + 1)],
                                         qpad[:, 128 * c0:],
                                         start=True, stop=True)
                        ptile = ptpool.tile([128, 512], F32R, tag="ptile")
                        nc.scalar.activation(
                            ptile[:, 128 * c0:], pt_ps[:, 128 * c0:],
                            mybir.ActivationFunctionType.Exp, scale=0.125)
                        for j in range(c0, 4):
                            jsl = slice(128 * j, 128 * (j + 1))
                            if sub[j] == 0:
                                nc.vector.tensor_copy(ptile[:, jsl],
                                                      zeros[:, 0:128])
                            elif sub[j] >= 2:
                                nc.vector.tensor_mul(
                                    ptile[:, jsl], ptile[:, jsl],
                                    mask_sb[:, sub[j] - 2, :])
                        if pending is not None:
                            flush(stop=False)
                        pending = (kb, ptile, c0)
                    flush(stop=True)

                    rc = rcpool.tile([1, 512], F32, tag="rc")
                    nc.vector.reciprocal(rc, av[DK:DK + 1, :])
                    rcr = rcpool.tile([1, 512], F32R, tag="rcr")
                    nc.vector.tensor_copy(rcr, rc)
                    bc_ps = psBC.tile([128, 512], F32, tag="bc")
                    nc.tensor.matmul(bc_ps, ones[:, 0:128], rcr,
                                     start=True, stop=True)
                    bcs = bcpool.tile([DK, 512], F32, tag="bcs")
                    nc.scalar.copy(bcs, bc_ps[0:DK, :])
                    nc.vector.tensor_mul(AT[hf:hf + DK, ob, qsl],
                                         av[0:DK, :], bcs)

                # out-projection for this q-tile
                for s2 in range(4):
                    sb = 4 * qt + s2
                    for ct in range(2):
                        csl = slice(512 * ct, 512 * (ct + 1))
                        po = psO.tile([128, 512], F32, tag="po")
                        for hb in range(4):
                            nc.tensor.matmul(
                                po, AT[:, hb, 128 * sb:128 * (sb + 1)],
                                wo_sb[:, hb, csl],
                                start=(hb == 0), stop=(hb == 3))
                        osb = opool.tile([128, 512], F32, tag="osb")
                        nc.vector.tensor_copy(osb, po)
                        nc.sync.dma_start(
                            out=y.rearrange("(b p) c -> b p c", p=128)[sb][:, csl],
                            in_=osb)

    nc.compile()
    return nc


def _classify_mask(mask2d):
    """Return (cls 16x16 list, mask_tiles list, causal flag) for the T
    orientation: cls[kb][qb] over 128x128 blocks of mask2d[q, k]."""
    m = (np.asarray(mask2d) != 0)
    blocks = m.reshape(NB, 128, NB, 128)  # [qb, ql, kb, kl]
    cls = [[0] * NB for _ in range(NB)]
    tiles = []
    keys = {}
    for kb in range(NB):
        for qb in range(NB):
            blk = blocks[qb, :, kb, :]  # [ql, kl]
            s = int(blk.sum())
            if s == 0:
                cls[kb][qb] = 0
            elif s == 128 * 128:
                cls[kb][qb] = 1
            else:
                t = np.ascontiguousarray(blk.T).astype(np.float32)  # [kl, ql]
                key = t.tobytes()
                if key not in keys:
                    keys[key] = len(tiles)
                    tiles.append(t)
                cls[kb][qb] = 2 + keys[key]
    causal = bool(np.array_equal(m, np.tril(np.ones((S, S), bool))))
    return cls, tiles, causal


_PROGRAM_CACHE = {}


def _get_program(mask2d):
    cls, tiles, causal = _classify_mask(mask2d)
    key = (tuple(tuple(r) for r in cls),
           tuple(t.tobytes() for t in tiles), causal)
    if key not in _PROGRAM_CACHE:
        _PROGRAM_CACHE[key] = (build_program(cls, tiles, causal), tiles, causal)
    return _PROGRAM_CACHE[key]


def run(inputs, trace=False):
    query = np.asarray(inputs["query"], np.float32)
    key_ = np.asarray(inputs["key"], np.float32)
    value = np.asarray(inputs["value"], np.float32)
    mask = np.asarray(inputs["mask"])
    Wq = np.asarray(inputs["Wq"], np.float32)
    bq = np.asarray(inputs["bq"], np.float32)
    Wk = np.asarray(inputs["Wk"], np.float32)
    bk = np.asarray(inputs["bk"], np.float32)
    Wv = np.asarray(inputs["Wv"], np.float32)
    bv = np.asarray(inputs["bv"], np.float32)
    Wo = np.asarray(inputs["Wo"], np.float32)
    bo = np.asarray(inputs["bo"], np.float32)

    nc, tiles, causal_flag = _get_program(mask[0, 0])

    in_maps = []
    for core in range(N_CORES):
        b, hg = core // 2, core % 2
        osl = slice(OC * hg, OC * (hg + 1))
        im = {
            "xqT": np.ascontiguousarray(query[b].T),
            "xkT": np.ascontiguousarray(key_[b].T),
            "xvT": np.ascontiguousarray(value[b].T),
            "wqT": np.ascontiguousarray(Wq.T[:, osl]),
            "wkT": np.ascontiguousarray(Wk.T[:, osl]),
            "wvT": np.ascontiguousarray(Wv.T[:, osl]),
            "bq": bq[osl].copy(),
            "bk": bk[osl].copy(),
            "bv": bv[osl].copy(),
            "woT": np.ascontiguousarray(Wo.T[osl, :]),
            "ones_row": np.ones(512, np.float32),
            "consts_va": np.concatenate(
                [np.ones((128, NB, HPC), np.float32),
                 np.zeros((128, NB, 64), np.float32)], axis=2),
            "zeros": np.zeros((128, 512), np.float32),
        }
        if tiles:
            im["masks"] = np.stack(tiles)
        in_maps.append(im)

    res = run_bass_kernel_spmd(nc, in_maps, list(range(N_CORES)), trace=trace)
    out = np.empty((B, S, D), np.float32)
    for b in range(B):
        out[b] = res.results[2 * b]["y"] + res.results[2 * b + 1]["y"]
    out += bo
    return out, res


def kernel(**inputs):
    out, _ = run(inputs, trace=False)
    return out


# revision 10
# speedup vs baseline: 1.2010x; 1.0154x over previous
"""Multi-head attention (B=4, S=2048, D=1024, H=16, causal) on 8 trn2 NeuronCores.

Sharding: core i handles batch b = i//2 and head-group hg = i%2 (8 heads each).
Data-parallel over B, tensor-parallel over heads; the out-projection partial
sums of the two head-groups of a batch are reduced on the host. No collectives.

Per-core dataflow (all matmuls in float32r, typed end-to-end — the BIR
verifier requires fp32r matmul operands to be produced as fp32r):
  phase A: QT[o,s], KT[o,s] (head-transposed) and V[s,o] (natural, augmented
           with a ones-column per head) via projections from host-transposed
           activations; biases folded in with K=1 augmented matmuls.
  phase B: per (q-tile, head): scores T[k,q] = KT_blk @ QT_tile on PE,
           exp on ScalarE (no max subtraction; causal scores are O(+-6)),
           causal masking via block skipping + one triangular mask tile,
           P.T @ [V|1] accumulation gives attention output (transposed) and
           softmax denominators in one PSUM tile; normalize via PE-broadcast
           of reciprocal denominators.
  phase C: out-projection with A.T blocks as stationary operands; per-core
           partial y (bo added on host).
"""

import numpy as np
from contextlib import ExitStack

import concourse.bass as bass
import concourse.tile as tile
from concourse import bacc, mybir
from concourse.bass_utils import run_bass_kernel_spmd

F32 = mybir.dt.float32
F32R = mybir.dt.float32r

B, S, D, H, DK = 4, 2048, 1024, 16, 64
HPC = 8          # heads per core
OC = HPC * DK    # 512 out-cols per core
NB = S // 128    # 16 seq blocks of 128
NQT = S // 512   # 4 q-tiles of 512
N_CORES = 8
VSTRIDE = DK + 1  # V cols per head incl the ones column


def build_program(cls, mask_tiles, causal):
    """cls[kb][qb] for the 16x16 grid of 128x128 blocks (T orientation:
    kb = key block, qb = query block): 0 = fully masked, 1 = fully valid,
    >=2 -> mixed, multiply by mask_tiles[cls-2] after exp."""
    n_mask = len(mask_tiles)
    nc = bacc.Bacc("TRN2", target_bir_lowering=False, debug=False,
                   num_devices=N_CORES, enable_asserts=False)

    xqT = nc.dram_tensor("xqT", [D, S], F32R, kind="ExternalInput").ap()
    xkT = nc.dram_tensor("xkT", [D, S], F32R, kind="ExternalInput").ap()
    xvT = nc.dram_tensor("xvT", [D, S], F32R, kind="ExternalInput").ap()
    wqT = nc.dram_tensor("wqT", [D, OC], F32R, kind="ExternalInput").ap()
    wkT = nc.dram_tensor("wkT", [D, OC], F32R, kind="ExternalInput").ap()
    wvT = nc.dram_tensor("wvT", [D, OC], F32R, kind="ExternalInput").ap()
    bq = nc.dram_tensor("bq", [OC], F32, kind="ExternalInput").ap()
    bk = nc.dram_tensor("bk", [OC], F32, kind="ExternalInput").ap()
    bv = nc.dram_tensor("bv", [OC], F32R, kind="ExternalInput").ap()
    woT = nc.dram_tensor("woT", [OC, D], F32R, kind="ExternalInput").ap()
    ones_row_d = nc.dram_tensor("ones_row", [512], F32R,
                                kind="ExternalInput").ap()
    vinit_d = nc.dram_tensor("vinit", [128, NB, HPC * VSTRIDE + 64], F32R,
                             kind="ExternalInput").ap()
    zeros_d = nc.dram_tensor("zeros", [128, 512], F32R,
                             kind="ExternalInput").ap()
    masks = None
    if n_mask:
        masks = nc.dram_tensor("masks", [n_mask, 128, 128], F32R,
                               kind="ExternalInput").ap()
    y = nc.dram_tensor("y", [S, D], F32, kind="ExternalOutput").ap()

    with tile.TileContext(nc) as tc, ExitStack() as ctx:
        persist = ctx.enter_context(tc.tile_pool(name="persist", bufs=1))
        QT = persist.tile([128, 4, S], F32R, tag="QT")
        KT = persist.tile([128, 4, S], F32R, tag="KT")
        # VSTRIDE*HPC data cols + 64 zero pad cols so the AV stationary can
        # always be a full [128, 128] window (M=128 is the fast LDW path)
        VA = persist.tile([128, NB, HPC * VSTRIDE + 64], F32R, tag="VA")
        ones = persist.tile([1, 512], F32R, tag="ones")
        nc.sync.dma_start(out=ones, in_=ones_row_d[None, :])
        zeros = None
        if not causal:
            zeros = persist.tile([128, 512], F32R, tag="zeros")
            nc.sync.dma_start(out=zeros, in_=zeros_d)
        # Q staging tiles, zero-padded so the scores matmul can contract over
        # the full 128 partitions (two-head KT block x one-head padded Q).
        # Even h uses qpad0 (head rows 0:64), odd h uses qpad1 (rows 64:128);
        # the other half of each stays zero forever.
        qpad0 = persist.tile([128, 512], F32R, tag="qpad0")
        qpad1 = persist.tile([128, 512], F32R, tag="qpad1")
        nc.sync.dma_start(out=qpad0, in_=zeros_d)
        nc.sync.dma_start(out=qpad1, in_=zeros_d)
        bq_pp = persist.tile([128, 4], F32, tag="bqp")
        bk_pp = persist.tile([128, 4], F32, tag="bkp")
        bv_sb = persist.tile([1, OC], F32R, tag="bv")
        nc.sync.dma_start(out=bq_pp, in_=bq.rearrange("(ob p) -> p ob", p=128))
        nc.sync.dma_start(out=bk_pp, in_=bk.rearrange("(ob p) -> p ob", p=128))
        nc.sync.dma_start(out=bv_sb, in_=bv[None, :])
        mask_sb = None
        if n_mask:
            mask_sb = persist.tile([128, n_mask, 128], F32R, tag="mask")
            nc.sync.dma_start(out=mask_sb, in_=masks.rearrange("m p j -> p m j"))
        # ones columns + zero pad, one efficient full-plane DMA (the V data
        # region is overwritten by the projection copies afterwards)
        nc.sync.dma_start(out=VA, in_=vinit_d)

        # ---------------- phase A: projections ----------------
        with tc.tile_pool(name="phA_w", bufs=1) as wpool, \
             tc.tile_pool(name="phA_x", bufs=2) as xpool, \
             tc.tile_pool(name="phA_ps", bufs=4, space=bass.MemorySpace.PSUM) as pps:
            wq_sb = wpool.tile([128, 8, OC], F32R, tag="wq")
            wk_sb = wpool.tile([128, 8, OC], F32R, tag="wk")
            wv_sb = wpool.tile([128, 8, OC], F32R, tag="wv")
            nc.sync.dma_start(out=wq_sb, in_=wqT.rearrange("(c p) o -> p c o", p=128))

            for which in ("q", "k", "v"):
                if which == "k":
                    nc.sync.dma_start(
                        out=wk_sb, in_=wkT.rearrange("(c p) o -> p c o", p=128))
                if which == "v":
                    nc.sync.dma_start(
                        out=wv_sb, in_=wvT.rearrange("(c p) o -> p c o", p=128))
                for st in range(NQT):
                    ssl = slice(512 * st, 512 * (st + 1))
                    xsrc = {"q": xqT, "k": xkT, "v": xvT}[which]
                    xs = xpool.tile([128, 8, 512], F32R, tag="xstage")
                    nc.sync.dma_start(
                        out=xs,
                        in_=xsrc.rearrange("(c p) s -> p c s", p=128)[:, :, ssl])
                    if which in ("q", "k"):
                        wsb = wq_sb if which == "q" else wk_sb
                        bpp = bq_pp if which == "q" else bk_pp
                        dst = QT if which == "q" else KT
                        for ob in range(4):
                            osl = slice(128 * ob, 128 * (ob + 1))
                            ps = pps.tile([128, 512], F32, tag="ps")
                            for c in range(8):
                                nc.tensor.matmul(ps, wsb[:, c, osl], xs[:, c, :],
                                                 start=(c == 0),
                                                 stop=(c == 7))
                            nc.vector.tensor_scalar_add(
                                dst[:, ob, ssl], ps, bpp[:, ob:ob + 1])
                    else:
                        for s2 in range(4):
                            sb = 4 * st + s2
                            ps = pps.tile([128, 512], F32, tag="ps")
                            for c in range(8):
                                nc.tensor.matmul(
                                    ps, xs[:, c, 128 * s2:128 * (s2 + 1)],
                                    wv_sb[:, c, :],
                                    start=(c == 0), stop=False)
                            nc.tensor.matmul(ps, ones[:, 0:128], bv_sb,
                                             start=False, stop=True)
                            nc.vector.tensor_copy(
                                VA[:, sb, 0:HPC * VSTRIDE].rearrange(
                                    "p (h e) -> p h e",
                                    e=VSTRIDE)[:, :, 0:DK],
                                ps.rearrange("p (h e) -> p h e", e=DK))

        # ---------------- phases B + C ----------------
        with tc.tile_pool(name="phB", bufs=1) as bpool, \
             tc.tile_pool(name="pt", bufs=3) as ptpool, \
             tc.tile_pool(name="bcs", bufs=2) as bcpool, \
             tc.tile_pool(name="rc", bufs=2) as rcpool, \
             tc.tile_pool(name="outst", bufs=3) as opool, \
             tc.tile_pool(name="psT", bufs=2, space=bass.MemorySpace.PSUM) as psT, \
             tc.tile_pool(name="psAV", bufs=2, space=bass.MemorySpace.PSUM) as psAV, \
             tc.tile_pool(name="psBC", bufs=2, space=bass.MemorySpace.PSUM) as psBC, \
             tc.tile_pool(name="psO", bufs=2, space=bass.MemorySpace.PSUM) as psO:
            AT = bpool.tile([128, 4, S], F32R, tag="AT")
            wo_sb = bpool.tile([128, 4, D], F32R, tag="wo")
            nc.sync.dma_start(out=wo_sb, in_=woT.rearrange("(c p) n -> p c n", p=128))

            norm_pending = None

            def norm_flush():
                av_, rcr_, hf_, ob_, qsl_ = norm_pending
                bc_ps = psBC.tile([128, 512], F32, tag="bc")
                nc.tensor.matmul(bc_ps, ones[:, 0:128], rcr_,
                                 start=True, stop=True)
                bcs = bcpool.tile([DK, 512], F32, tag="bcs")
                nc.scalar.copy(bcs, bc_ps[0:DK, :])
                nc.vector.tensor_mul(AT[hf_:hf_ + DK, ob_, qsl_],
                                     av_[0:DK, :], bcs)

            for qt in range(NQT):
                qsl = slice(512 * qt, 512 * (qt + 1))
                for h in range(HPC):
                    ob, hf = h // 2, (h % 2) * DK
                    qpad = qpad0 if h % 2 == 0 else qpad1
                    nc.vector.tensor_copy(qpad[hf:hf + DK, :],
                                          QT[hf:hf + DK, ob, qsl])
                    vsl = slice(VSTRIDE * h, VSTRIDE * h + 128)
                    active = [kb for kb in range(NB)
                              if any(cls[kb][4 * qt + j] for j in range(4))]
                    if not active:
                        nc.vector.tensor_copy(AT[hf:hf + DK, ob, qsl],
                                              zeros[0:DK, :])
                        continue
                    av = psAV.tile([128, 512], F32, tag="av")
                    pending = None  # (kb, ptile, c0) awaiting its AV matmul

                    def flush(stop):
                        kb_, pt_, c0_ = pending
                        nc.tensor.matmul(
                            av[:, 128 * c0_:], VA[:, kb_, vsl],
                            pt_[:, 128 * c0_:],
                            start=(kb_ == active[0]), stop=stop)

                    for kb in active:
                        sub = [cls[kb][4 * qt + j] for j in range(4)]
                        if causal:
                            c0 = kb - 4 * qt if kb >= 4 * qt else 0
                        else:
                            c0 = 0
                        if kb == active[0]:
                            c0 = 0  # first AV matmul must cover all columns
                        pt_ps = psT.tile([128, 512], F32, tag="pt")
                        nc.tensor.matmul(pt_ps[:, 128 * c0:],
                                         KT[:, ob, 128 * kb:128 * (kb + 1)],
                                         qpad[:, 128 * c0:],
                                         start=True, stop=True)
                        ptile = ptpool.tile([128, 512], F32R, tag="ptile")
                        nc.scalar.activation(
                            ptile[:, 128 * c0:], pt_ps[:, 128 * c0:],
                            mybir.ActivationFunctionType.Exp, scale=0.125)
                        for j in range(c0, 4):
                            jsl = slice(128 * j, 128 * (j + 1))
                            if sub[j] == 0:
                                nc.vector.tensor_copy(ptile[:, jsl],
                                                      zeros[:, 0:128])
                            elif sub[j] >= 2:
                                nc.vector.tensor_mul(
                                    ptile[:, jsl], ptile[:, jsl],
                                    mask_sb[:, sub[j] - 2, :])
                        if pending is not None:
                            flush(stop=False)
                        pending = (kb, ptile, c0)
                    flush(stop=True)

                    # reciprocal is ~3.3us on DVE; emit it now but defer the
                    # PE broadcast + normalize by one head so the PE never
                    # stalls waiting for it
                    rc = rcpool.tile([1, 512], F32, tag="rc")
                    nc.vector.reciprocal(rc, av[DK:DK + 1, :])
                    rcr = rcpool.tile([1, 512], F32R, tag="rcr")
                    nc.vector.tensor_copy(rcr, rc)
                    if norm_pending is not None:
                        norm_flush()
                    norm_pending = (av, rcr, hf, ob, qsl)
                if norm_pending is not None:
                    norm_flush()
                    norm_pending = None

                # out-projection for this q-tile
                for s2 in range(4):
                    sb = 4 * qt + s2
                    for ct in range(2):
                        csl = slice(512 * ct, 512 * (ct + 1))
                        po = psO.tile([128, 512], F32, tag="po")
                        for hb in range(4):
                            nc.tensor.matmul(
                                po, AT[:, hb, 128 * sb:128 * (sb + 1)],
                                wo_sb[:, hb, csl],
                                start=(hb == 0), stop=(hb == 3))
                        osb = opool.tile([128, 512], F32, tag="osb")
                        nc.vector.tensor_copy(osb, po)
                        nc.sync.dma_start(
                            out=y.rearrange("(b p) c -> b p c", p=128)[sb][:, csl],
                            in_=osb)

    nc.compile()
    return nc


def _classify_mask(mask2d):
    """Return (cls 16x16 list, mask_tiles list, causal flag) for the T
    orientation: cls[kb][qb] over 128x128 blocks of mask2d[q, k]."""
    m = (np.asarray(mask2d) != 0)
    blocks = m.reshape(NB, 128, NB, 128)  # [qb, ql, kb, kl]
    cls = [[0] * NB for _ in range(NB)]
    tiles = []
    keys = {}
    for kb in range(NB):
        for qb in range(NB):
            blk = blocks[qb, :, kb, :]  # [ql, kl]
            s = int(blk.sum())
            if s == 0:
                cls[kb][qb] = 0
            elif s == 128 * 128:
                cls[kb][qb] = 1
            else:
                t = np.ascontiguousarray(blk.T).astype(np.float32)  # [kl, ql]
                key = t.tobytes()
                if key not in keys:
                    keys[key] = len(tiles)
                    tiles.append(t)
                cls[kb][qb] = 2 + keys[key]
    causal = bool(np.array_equal(m, np.tril(np.ones((S, S), bool))))
    return cls, tiles, causal


def _vinit_plane():
    v = np.zeros((128, NB, HPC * VSTRIDE + 64), np.float32)
    for h in range(HPC):
        v[:, :, VSTRIDE * h + DK] = 1.0
    return v


_PROGRAM_CACHE = {}


def _get_program(mask2d):
    cls, tiles, causal = _classify_mask(mask2d)
    key = (tuple(tuple(r) for r in cls),
           tuple(t.tobytes() for t in tiles), causal)
    if key not in _PROGRAM_CACHE:
        _PROGRAM_CACHE[key] = (build_program(cls, tiles, causal), tiles, causal)
    return _PROGRAM_CACHE[key]


def run(inputs, trace=False):
    query = np.asarray(inputs["query"], np.float32)
    key_ = np.asarray(inputs["key"], np.float32)
    value = np.asarray(inputs["value"], np.float32)
    mask = np.asarray(inputs["mask"])
    Wq = np.asarray(inputs["Wq"], np.float32)
    bq = np.asarray(inputs["bq"], np.float32)
    Wk = np.asarray(inputs["Wk"], np.float32)
    bk = np.asarray(inputs["bk"], np.float32)
    Wv = np.asarray(inputs["Wv"], np.float32)
    bv = np.asarray(inputs["bv"], np.float32)
    Wo = np.asarray(inputs["Wo"], np.float32)
    bo = np.asarray(inputs["bo"], np.float32)

    nc, tiles, causal_flag = _get_program(mask[0, 0])

    in_maps = []
    for core in range(N_CORES):
        b, hg = core // 2, core % 2
        osl = slice(OC * hg, OC * (hg + 1))
        im = {
            "xqT": np.ascontiguousarray(query[b].T),
            "xkT": np.ascontiguousarray(key_[b].T),
            "xvT": np.ascontiguousarray(value[b].T),
            "wqT": np.ascontiguousarray(Wq.T[:, osl]),
            "wkT": np.ascontiguousarray(Wk.T[:, osl]),
            "wvT": np.ascontiguousarray(Wv.T[:, osl]),
            "bq": bq[osl].copy(),
            "bk": bk[osl].copy(),
            "bv": bv[osl].copy(),
            "woT": np.ascontiguousarray(Wo.T[osl, :]),
            "ones_row": np.ones(512, np.float32),
            "vinit": _vinit_plane(),
            "zeros": np.zeros((128, 512), np.float32),
        }
        if tiles:
            im["masks"] = np.stack(tiles)
        in_maps.append(im)

    res = run_bass_kernel_spmd(nc, in_maps, list(range(N_CORES)), trace=trace)
    out = np.empty((B, S, D), np.float32)
    for b in range(B):
        out[b] = res.results[2 * b]["y"] + res.results[2 * b + 1]["y"]
    out += bo
    return out, res


def kernel(**inputs):
    out, _ = run(inputs, trace=False)
    return out


# revision 12
# speedup vs baseline: 1.2093x; 1.0069x over previous
"""Multi-head attention (B=4, S=2048, D=1024, H=16, causal) on 8 trn2 NeuronCores.

Sharding: core i handles batch b = i//2 and head-group hg = i%2 (8 heads each).
Data-parallel over B, tensor-parallel over heads; the out-projection partial
sums of the two head-groups of a batch are reduced on the host. No collectives.

Per-core dataflow (all matmuls in float32r, typed end-to-end — the BIR
verifier requires fp32r matmul operands to be produced as fp32r):
  phase A: QT[o,s], KT[o,s] (head-transposed) and V[s,o] (natural, augmented
           with a ones-column per head) via projections from host-transposed
           activations; biases folded in with K=1 augmented matmuls.
  phase B: per (q-tile, head): scores T[k,q] = KT_blk @ QT_tile on PE,
           exp on ScalarE (no max subtraction; causal scores are O(+-6)),
           causal masking via block skipping + one triangular mask tile,
           P.T @ [V|1] accumulation gives attention output (transposed) and
           softmax denominators in one PSUM tile; normalize via PE-broadcast
           of reciprocal denominators.
  phase C: out-projection with A.T blocks as stationary operands; per-core
           partial y (bo added on host).
"""

import numpy as np
from contextlib import ExitStack

import concourse.bass as bass
import concourse.tile as tile
from concourse import bacc, mybir
from concourse.bass_utils import run_bass_kernel_spmd

F32 = mybir.dt.float32
F32R = mybir.dt.float32r

B, S, D, H, DK = 4, 2048, 1024, 16, 64
HPC = 8          # heads per core
OC = HPC * DK    # 512 out-cols per core
NB = S // 128    # 16 seq blocks of 128
NQT = S // 512   # 4 q-tiles of 512
N_CORES = 8
VSTRIDE = DK + 1  # V cols per head incl the ones column


def build_program(cls, mask_tiles, causal):
    """cls[kb][qb] for the 16x16 grid of 128x128 blocks (T orientation:
    kb = key block, qb = query block): 0 = fully masked, 1 = fully valid,
    >=2 -> mixed, multiply by mask_tiles[cls-2] after exp."""
    n_mask = len(mask_tiles)
    nc = bacc.Bacc("TRN2", target_bir_lowering=False, debug=False,
                   num_devices=N_CORES, enable_asserts=False)

    xqT = nc.dram_tensor("xqT", [D, S], F32R, kind="ExternalInput").ap()
    xkT = nc.dram_tensor("xkT", [D, S], F32R, kind="ExternalInput").ap()
    xvT = nc.dram_tensor("xvT", [D, S], F32R, kind="ExternalInput").ap()
    wqT = nc.dram_tensor("wqT", [D, OC], F32R, kind="ExternalInput").ap()
    wkT = nc.dram_tensor("wkT", [D, OC], F32R, kind="ExternalInput").ap()
    wvT = nc.dram_tensor("wvT", [D, OC], F32R, kind="ExternalInput").ap()
    bq = nc.dram_tensor("bq", [OC], F32, kind="ExternalInput").ap()
    bk = nc.dram_tensor("bk", [OC], F32, kind="ExternalInput").ap()
    bv = nc.dram_tensor("bv", [OC], F32R, kind="ExternalInput").ap()
    woT = nc.dram_tensor("woT", [OC, D], F32R, kind="ExternalInput").ap()
    ones_row_d = nc.dram_tensor("ones_row", [512], F32R,
                                kind="ExternalInput").ap()
    vinit_d = nc.dram_tensor("vinit", [128, NB, HPC * VSTRIDE + 64], F32R,
                             kind="ExternalInput").ap()
    zeros_d = nc.dram_tensor("zeros", [128, 512], F32R,
                             kind="ExternalInput").ap()
    masks = None
    if n_mask:
        masks = nc.dram_tensor("masks", [n_mask, 128, 128], F32R,
                               kind="ExternalInput").ap()
    y = nc.dram_tensor("y", [S, D], F32, kind="ExternalOutput").ap()

    with tile.TileContext(nc) as tc, ExitStack() as ctx:
        persist = ctx.enter_context(tc.tile_pool(name="persist", bufs=1))
        QT = persist.tile([128, 4, S], F32R, tag="QT")
        KT = persist.tile([128, 4, S], F32R, tag="KT")
        # VSTRIDE*HPC data cols + 64 zero pad cols so the AV stationary can
        # always be a full [128, 128] window (M=128 is the fast LDW path)
        VA = persist.tile([128, NB, HPC * VSTRIDE + 64], F32R, tag="VA")
        ones = persist.tile([1, 512], F32R, tag="ones")
        nc.sync.dma_start(out=ones, in_=ones_row_d[None, :])
        zeros = None
        if not causal:
            zeros = persist.tile([128, 512], F32R, tag="zeros")
            nc.sync.dma_start(out=zeros, in_=zeros_d)
        # Q staging tiles, zero-padded so the scores matmul can contract over
        # the full 128 partitions (two-head KT block x one-head padded Q).
        # Even h uses qpad0 (head rows 0:64), odd h uses qpad1 (rows 64:128);
        # the other half of each stays zero forever.
        qpad0 = persist.tile([128, 512], F32R, tag="qpad0")
        qpad1 = persist.tile([128, 512], F32R, tag="qpad1")
        nc.sync.dma_start(out=qpad0, in_=zeros_d)
        nc.sync.dma_start(out=qpad1, in_=zeros_d)
        bq_pp = persist.tile([128, 4], F32, tag="bqp")
        bk_pp = persist.tile([128, 4], F32, tag="bkp")
        bv_sb = persist.tile([1, OC], F32R, tag="bv")
        nc.sync.dma_start(out=bq_pp, in_=bq.rearrange("(ob p) -> p ob", p=128))
        nc.sync.dma_start(out=bk_pp, in_=bk.rearrange("(ob p) -> p ob", p=128))
        nc.sync.dma_start(out=bv_sb, in_=bv[None, :])
        mask_sb = None
        if n_mask:
            mask_sb = persist.tile([128, n_mask, 128], F32R, tag="mask")
            nc.sync.dma_start(out=mask_sb, in_=masks.rearrange("m p j -> p m j"))
        # ones columns + zero pad, one efficient full-plane DMA (the V data
        # region is overwritten by the projection copies afterwards)
        nc.sync.dma_start(out=VA, in_=vinit_d)

        # ---------------- phase A: projections ----------------
        with tc.tile_pool(name="phA_w", bufs=1) as wpool, \
             tc.tile_pool(name="phA_x", bufs=2) as xpool, \
             tc.tile_pool(name="phA_ps", bufs=4, space=bass.MemorySpace.PSUM) as pps:
            wq_sb = wpool.tile([128, 8, OC], F32R, tag="wq")
            wk_sb = wpool.tile([128, 8, OC], F32R, tag="wk")
            wv_sb = wpool.tile([128, 8, OC], F32R, tag="wv")
            nc.sync.dma_start(out=wq_sb, in_=wqT.rearrange("(c p) o -> p c o", p=128))

            for which in ("q", "k", "v"):
                if which == "k":
                    nc.sync.dma_start(
                        out=wk_sb, in_=wkT.rearrange("(c p) o -> p c o", p=128))
                if which == "v":
                    nc.sync.dma_start(
                        out=wv_sb, in_=wvT.rearrange("(c p) o -> p c o", p=128))
                for st in range(NQT):
                    ssl = slice(512 * st, 512 * (st + 1))
                    xsrc = {"q": xqT, "k": xkT, "v": xvT}[which]
                    xs = xpool.tile([128, 8, 512], F32R, tag="xstage")
                    nc.sync.dma_start(
                        out=xs,
                        in_=xsrc.rearrange("(c p) s -> p c s", p=128)[:, :, ssl])
                    if which in ("q", "k"):
                        wsb = wq_sb if which == "q" else wk_sb
                        bpp = bq_pp if which == "q" else bk_pp
                        dst = QT if which == "q" else KT
                        for ob in range(4):
                            osl = slice(128 * ob, 128 * (ob + 1))
                            ps = pps.tile([128, 512], F32, tag="ps")
                            for c in range(8):
                                nc.tensor.matmul(ps, wsb[:, c, osl], xs[:, c, :],
                                                 start=(c == 0),
                                                 stop=(c == 7))
                            nc.vector.tensor_scalar_add(
                                dst[:, ob, ssl], ps, bpp[:, ob:ob + 1])
                    else:
                        for s2 in range(4):
                            sb = 4 * st + s2
                            ps = pps.tile([128, 512], F32, tag="ps")
                            for c in range(8):
                                nc.tensor.matmul(
                                    ps, xs[:, c, 128 * s2:128 * (s2 + 1)],
                                    wv_sb[:, c, :],
                                    start=(c == 0), stop=False)
                            nc.tensor.matmul(ps, ones[:, 0:128], bv_sb,
                                             start=False, stop=True)
                            nc.vector.tensor_copy(
                                VA[:, sb, 0:HPC * VSTRIDE].rearrange(
                                    "p (h e) -> p h e",
                                    e=VSTRIDE)[:, :, 0:DK],
                                ps.rearrange("p (h e) -> p h e", e=DK))

        # ---------------- phases B + C ----------------
        with tc.tile_pool(name="phB", bufs=1) as bpool, \
             tc.tile_pool(name="pt", bufs=3) as ptpool, \
             tc.tile_pool(name="bcs", bufs=2) as bcpool, \
             tc.tile_pool(name="rc", bufs=2) as rcpool, \
             tc.tile_pool(name="outst", bufs=3) as opool, \
             tc.tile_pool(name="psT", bufs=2, space=bass.MemorySpace.PSUM) as psT, \
             tc.tile_pool(name="psAV", bufs=2, space=bass.MemorySpace.PSUM) as psAV, \
             tc.tile_pool(name="psBC", bufs=2, space=bass.MemorySpace.PSUM) as psBC, \
             tc.tile_pool(name="psO", bufs=2, space=bass.MemorySpace.PSUM) as psO:
            AT = bpool.tile([128, 4, S], F32R, tag="AT")
            wo_sb = bpool.tile([128, 4, D], F32R, tag="wo")
            nc.sync.dma_start(out=wo_sb, in_=woT.rearrange("(c p) n -> p c n", p=128))

            norm_pending = None

            def norm_flush():
                av_, hf_, ob_, qsl_ = norm_pending
                rc = rcpool.tile([1, 512], F32, tag="rc")
                nc.vector.reciprocal(rc, av_[DK:DK + 1, :])
                rcr = rcpool.tile([1, 512], F32R, tag="rcr")
                nc.vector.tensor_copy(rcr, rc)
                bc_ps = psBC.tile([128, 512], F32, tag="bc")
                nc.tensor.matmul(bc_ps, ones[:, 0:128], rcr,
                                 start=True, stop=True)
                bcs = bcpool.tile([DK, 512], F32, tag="bcs")
                nc.scalar.copy(bcs, bc_ps[0:DK, :])
                nc.vector.tensor_mul(AT[hf_:hf_ + DK, ob_, qsl_],
                                     av_[0:DK, :], bcs)

            for qt in range(NQT):
                qsl = slice(512 * qt, 512 * (qt + 1))
                for h in range(HPC):
                    ob, hf = h // 2, (h % 2) * DK
                    qpad = qpad0 if h % 2 == 0 else qpad1
                    nc.vector.tensor_copy(qpad[hf:hf + DK, :],
                                          QT[hf:hf + DK, ob, qsl])
                    vsl = slice(VSTRIDE * h, VSTRIDE * h + 128)
                    active = [kb for kb in range(NB)
                              if any(cls[kb][4 * qt + j] for j in range(4))]
                    if not active:
                        nc.vector.tensor_copy(AT[hf:hf + DK, ob, qsl],
                                              zeros[0:DK, :])
                        continue
                    av = psAV.tile([128, 512], F32, tag="av")
                    pending = None  # (kb, ptile, c0) awaiting its AV matmul

                    def flush(stop):
                        kb_, pt_, c0_ = pending
                        nc.tensor.matmul(
                            av[:, 128 * c0_:], VA[:, kb_, vsl],
                            pt_[:, 128 * c0_:],
                            start=(kb_ == active[0]), stop=stop)

                    for kb in active:
                        sub = [cls[kb][4 * qt + j] for j in range(4)]
                        if causal:
                            c0 = kb - 4 * qt if kb >= 4 * qt else 0
                        else:
                            c0 = 0
                        if kb == active[0]:
                            c0 = 0  # first AV matmul must cover all columns
                        pt_ps = psT.tile([128, 512], F32, tag="pt")
                        nc.tensor.matmul(pt_ps[:, 128 * c0:],
                                         KT[:, ob, 128 * kb:128 * (kb + 1)],
                                         qpad[:, 128 * c0:],
                                         start=True, stop=True)
                        ptile = ptpool.tile([128, 512], F32R, tag="ptile")
                        nc.scalar.activation(
                            ptile[:, 128 * c0:], pt_ps[:, 128 * c0:],
                            mybir.ActivationFunctionType.Exp, scale=0.125)
                        for j in range(c0, 4):
                            jsl = slice(128 * j, 128 * (j + 1))
                            if sub[j] == 0:
                                nc.vector.tensor_copy(ptile[:, jsl],
                                                      zeros[:, 0:128])
                            elif sub[j] >= 2:
                                nc.vector.tensor_mul(
                                    ptile[:, jsl], ptile[:, jsl],
                                    mask_sb[:, sub[j] - 2, :])
                        if pending is not None:
                            flush(stop=False)
                        pending = (kb, ptile, c0)
                    flush(stop=True)

                    # the ~3.3us DVE reciprocal and the whole normalize
                    # chain run one head behind, so neither PE nor the next
                    # head's DVE staging waits on them
                    if norm_pending is not None:
                        norm_flush()
                    norm_pending = (av, hf, ob, qsl)
                if norm_pending is not None:
                    norm_flush()
                    norm_pending = None

                # out-projection for this q-tile
                for s2 in range(4):
                    sb = 4 * qt + s2
                    for ct in range(2):
                        csl = slice(512 * ct, 512 * (ct + 1))
                        po = psO.tile([128, 512], F32, tag="po")
                        for hb in range(4):
                            nc.tensor.matmul(
                                po, AT[:, hb, 128 * sb:128 * (sb + 1)],
                                wo_sb[:, hb, csl],
                                start=(hb == 0), stop=(hb == 3))
                        osb = opool.tile([128, 512], F32, tag="osb")
                        nc.vector.tensor_copy(osb, po)
                        nc.sync.dma_start(
                            out=y.rearrange("(b p) c -> b p c", p=128)[sb][:, csl],
                            in_=osb)

    nc.compile()
    return nc


def _classify_mask(mask2d):
    """Return (cls 16x16 list, mask_tiles list, causal flag) for the T
    orientation: cls[kb][qb] over 128x128 blocks of mask2d[q, k]."""
    m = (np.asarray(mask2d) != 0)
    blocks = m.reshape(NB, 128, NB, 128)  # [qb, ql, kb, kl]
    cls = [[0] * NB for _ in range(NB)]
    tiles = []
    keys = {}
    for kb in range(NB):
        for qb in range(NB):
            blk = blocks[qb, :, kb, :]  # [ql, kl]
            s = int(blk.sum())
            if s == 0:
                cls[kb][qb] = 0
            elif s == 128 * 128:
                cls[kb][qb] = 1
            else:
                t = np.ascontiguousarray(blk.T).astype(np.float32)  # [kl, ql]
                key = t.tobytes()
                if key not in keys:
                    keys[key] = len(tiles)
                    tiles.append(t)
                cls[kb][qb] = 2 + keys[key]
    causal = bool(np.array_equal(m, np.tril(np.ones((S, S), bool))))
    return cls, tiles, causal


def _vinit_plane():
    v = np.zeros((128, NB, HPC * VSTRIDE + 64), np.float32)
    for h in range(HPC):
        v[:, :, VSTRIDE * h + DK] = 1.0
    return v


_PROGRAM_CACHE = {}


def _get_program(mask2d):
    cls, tiles, causal = _classify_mask(mask2d)
    key = (tuple(tuple(r) for r in cls),
           tuple(t.tobytes() for t in tiles), causal)
    if key not in _PROGRAM_CACHE:
        _PROGRAM_CACHE[key] = (build_program(cls, tiles, causal), tiles, causal)
    return _PROGRAM_CACHE[key]


def run(inputs, trace=False):
    query = np.asarray(inputs["query"], np.float32)
    key_ = np.asarray(inputs["key"], np.float32)
    value = np.asarray(inputs["value"], np.float32)
    mask = np.asarray(inputs["mask"])
    Wq = np.asarray(inputs["Wq"], np.float32)
    bq = np.asarray(inputs["bq"], np.float32)
    Wk = np.asarray(inputs["Wk"], np.float32)
    bk = np.asarray(inputs["bk"], np.float32)
    Wv = np.asarray(inputs["Wv"], np.float32)
    bv = np.asarray(inputs["bv"], np.float32)
    Wo = np.asarray(inputs["Wo"], np.float32)
    bo = np.asarray(inputs["bo"], np.float32)

    nc, tiles, causal_flag = _get_program(mask[0, 0])

    in_maps = []
    for core in range(N_CORES):
        b, hg = core // 2, core % 2
        osl = slice(OC * hg, OC * (hg + 1))
        im = {
            "xqT": np.ascontiguousarray(query[b].T),
            "xkT": np.ascontiguousarray(key_[b].T),
            "xvT": np.ascontiguousarray(value[b].T),
            "wqT": np.ascontiguousarray(Wq.T[:, osl]),
            "wkT": np.ascontiguousarray(Wk.T[:, osl]),
            "wvT": np.ascontiguousarray(Wv.T[:, osl]),
            "bq": bq[osl].copy(),
            "bk": bk[osl].copy(),
            "bv": bv[osl].copy(),
            "woT": np.ascontiguousarray(Wo.T[osl, :]),
            "ones_row": np.ones(512, np.float32),
            "vinit": _vinit_plane(),
            "zeros": np.zeros((128, 512), np.float32),
        }
        if tiles:
            im["masks"] = np.stack(tiles)
        in_maps.append(im)

    res = run_bass_kernel_spmd(nc, in_maps, list(range(N_CORES)), trace=trace)
    out = np.empty((B, S, D), np.float32)
    for b in range(B):
        out[b] = res.results[2 * b]["y"] + res.results[2 * b + 1]["y"]
    out += bo
    return out, res


def kernel(**inputs):
    out, _ = run(inputs, trace=False)
    return out


# revision 13
# speedup vs baseline: 1.2177x; 1.0069x over previous
"""Multi-head attention (B=4, S=2048, D=1024, H=16, causal) on 8 trn2 NeuronCores.

Sharding: core i handles batch b = i//2 and head-group hg = i%2 (8 heads each).
Data-parallel over B, tensor-parallel over heads; the out-projection partial
sums of the two head-groups of a batch are reduced on the host. No collectives.

Per-core dataflow (all matmuls in float32r, typed end-to-end — the BIR
verifier requires fp32r matmul operands to be produced as fp32r):
  phase A: QT[o,s], KT[o,s] (head-transposed) and V[s,o] (natural, augmented
           with a ones-column per head) via projections from host-transposed
           activations; biases folded in with K=1 augmented matmuls.
  phase B: per (q-tile, head): scores T[k,q] = KT_blk @ QT_tile on PE,
           exp on ScalarE (no max subtraction; causal scores are O(+-6)),
           causal masking via block skipping + one triangular mask tile,
           P.T @ [V|1] accumulation gives attention output (transposed) and
           softmax denominators in one PSUM tile; normalize via PE-broadcast
           of reciprocal denominators.
  phase C: out-projection with A.T blocks as stationary operands; per-core
           partial y (bo added on host).
"""

import numpy as np
from contextlib import ExitStack

import concourse.bass as bass
import concourse.tile as tile
from concourse import bacc, mybir
from concourse.bass_utils import run_bass_kernel_spmd

F32 = mybir.dt.float32
F32R = mybir.dt.float32r

B, S, D, H, DK = 4, 2048, 1024, 16, 64
HPC = 8          # heads per core
OC = HPC * DK    # 512 out-cols per core
NB = S // 128    # 16 seq blocks of 128
NQT = S // 512   # 4 q-tiles of 512
N_CORES = 8
VSTRIDE = DK + 1  # V cols per head incl the ones column


def build_program(cls, mask_tiles, causal):
    """cls[kb][qb] for the 16x16 grid of 128x128 blocks (T orientation:
    kb = key block, qb = query block): 0 = fully masked, 1 = fully valid,
    >=2 -> mixed, multiply by mask_tiles[cls-2] after exp."""
    n_mask = len(mask_tiles)
    nc = bacc.Bacc("TRN2", target_bir_lowering=False, debug=False,
                   num_devices=N_CORES, enable_asserts=False)

    xqT = nc.dram_tensor("xqT", [D, S], F32R, kind="ExternalInput").ap()
    xkT = nc.dram_tensor("xkT", [D, S], F32R, kind="ExternalInput").ap()
    xvT = nc.dram_tensor("xvT", [D, S], F32R, kind="ExternalInput").ap()
    wqT = nc.dram_tensor("wqT", [D, OC], F32R, kind="ExternalInput").ap()
    wkT = nc.dram_tensor("wkT", [D, OC], F32R, kind="ExternalInput").ap()
    wvT = nc.dram_tensor("wvT", [D, OC], F32R, kind="ExternalInput").ap()
    bq = nc.dram_tensor("bq", [OC], F32, kind="ExternalInput").ap()
    bk = nc.dram_tensor("bk", [OC], F32, kind="ExternalInput").ap()
    bv = nc.dram_tensor("bv", [OC], F32R, kind="ExternalInput").ap()
    woT = nc.dram_tensor("woT", [OC, D], F32R, kind="ExternalInput").ap()
    ones_row_d = nc.dram_tensor("ones_row", [512], F32R,
                                kind="ExternalInput").ap()
    vinit_d = nc.dram_tensor("vinit", [128, NB, HPC * VSTRIDE + 64], F32R,
                             kind="ExternalInput").ap()
    zeros_d = nc.dram_tensor("zeros", [128, 512], F32R,
                             kind="ExternalInput").ap()
    masks = None
    if n_mask:
        masks = nc.dram_tensor("masks", [n_mask, 128, 128], F32R,
                               kind="ExternalInput").ap()
    y = nc.dram_tensor("y", [S, D], F32, kind="ExternalOutput").ap()

    with tile.TileContext(nc) as tc, ExitStack() as ctx:
        persist = ctx.enter_context(tc.tile_pool(name="persist", bufs=1))
        QT = persist.tile([128, 4, S], F32R, tag="QT")
        KT = persist.tile([128, 4, S], F32R, tag="KT")
        # VSTRIDE*HPC data cols + 64 zero pad cols so the AV stationary can
        # always be a full [128, 128] window (M=128 is the fast LDW path)
        VA = persist.tile([128, NB, HPC * VSTRIDE + 64], F32R, tag="VA")
        ones = persist.tile([1, 512], F32R, tag="ones")
        nc.sync.dma_start(out=ones, in_=ones_row_d[None, :])
        zeros = None
        if not causal:
            zeros = persist.tile([128, 512], F32R, tag="zeros")
            nc.sync.dma_start(out=zeros, in_=zeros_d)
        # Q staging tiles, zero-padded so the scores matmul can contract over
        # the full 128 partitions (two-head KT block x one-head padded Q).
        # Even h uses qpad0 (head rows 0:64), odd h uses qpad1 (rows 64:128);
        # the other half of each stays zero forever.
        qpad0 = persist.tile([128, 512], F32R, tag="qpad0")
        qpad1 = persist.tile([128, 512], F32R, tag="qpad1")
        nc.sync.dma_start(out=qpad0, in_=zeros_d)
        nc.sync.dma_start(out=qpad1, in_=zeros_d)
        bq_pp = persist.tile([128, 4], F32, tag="bqp")
        bk_pp = persist.tile([128, 4], F32, tag="bkp")
        bv_sb = persist.tile([1, OC], F32R, tag="bv")
        nc.sync.dma_start(out=bq_pp, in_=bq.rearrange("(ob p) -> p ob", p=128))
        nc.sync.dma_start(out=bk_pp, in_=bk.rearrange("(ob p) -> p ob", p=128))
        nc.sync.dma_start(out=bv_sb, in_=bv[None, :])
        mask_sb = None
        if n_mask:
            mask_sb = persist.tile([128, n_mask, 128], F32R, tag="mask")
            nc.sync.dma_start(out=mask_sb, in_=masks.rearrange("m p j -> p m j"))
        # ones columns + zero pad, one efficient full-plane DMA (the V data
        # region is overwritten by the projection copies afterwards)
        nc.sync.dma_start(out=VA, in_=vinit_d)

        # ---------------- phase A: projections ----------------
        with tc.tile_pool(name="phA_w", bufs=1) as wpool, \
             tc.tile_pool(name="phA_x", bufs=2) as xpool, \
             tc.tile_pool(name="phA_ps", bufs=4, space=bass.MemorySpace.PSUM) as pps:
            wq_sb = wpool.tile([128, 8, OC], F32R, tag="wq")
            wk_sb = wpool.tile([128, 8, OC], F32R, tag="wk")
            wv_sb = wpool.tile([128, 8, OC], F32R, tag="wv")
            for c in range(8):
                nc.sync.dma_start(
                    out=wq_sb[:, c, :],
                    in_=wqT.rearrange("(c p) o -> p c o", p=128)[:, c, :])

            for which in ("q", "k", "v"):
                if which == "k":
                    for c in range(8):
                        nc.sync.dma_start(
                            out=wk_sb[:, c, :],
                            in_=wkT.rearrange("(c p) o -> p c o", p=128)[:, c, :])
                if which == "v":
                    for c in range(8):
                        nc.sync.dma_start(
                            out=wv_sb[:, c, :],
                            in_=wvT.rearrange("(c p) o -> p c o", p=128)[:, c, :])
                for st in range(NQT):
                    ssl = slice(512 * st, 512 * (st + 1))
                    xsrc = {"q": xqT, "k": xkT, "v": xvT}[which]
                    xs = xpool.tile([128, 8, 512], F32R, tag="xstage")
                    for c in range(8):
                        nc.sync.dma_start(
                            out=xs[:, c, :],
                            in_=xsrc.rearrange("(c p) s -> p c s",
                                               p=128)[:, c, ssl])
                    if which in ("q", "k"):
                        wsb = wq_sb if which == "q" else wk_sb
                        bpp = bq_pp if which == "q" else bk_pp
                        dst = QT if which == "q" else KT
                        for ob in range(4):
                            osl = slice(128 * ob, 128 * (ob + 1))
                            ps = pps.tile([128, 512], F32, tag="ps")
                            for c in range(8):
                                nc.tensor.matmul(ps, wsb[:, c, osl], xs[:, c, :],
                                                 start=(c == 0),
                                                 stop=(c == 7))
                            nc.vector.tensor_scalar_add(
                                dst[:, ob, ssl], ps, bpp[:, ob:ob + 1])
                    else:
                        for s2 in range(4):
                            sb = 4 * st + s2
                            ps = pps.tile([128, 512], F32, tag="ps")
                            for c in range(8):
                                nc.tensor.matmul(
                                    ps, xs[:, c, 128 * s2:128 * (s2 + 1)],
                                    wv_sb[:, c, :],
                                    start=(c == 0), stop=False)
                            nc.tensor.matmul(ps, ones[:, 0:128], bv_sb,
                                             start=False, stop=True)
                            nc.vector.tensor_copy(
                                VA[:, sb, 0:HPC * VSTRIDE].rearrange(
                                    "p (h e) -> p h e",
                                    e=VSTRIDE)[:, :, 0:DK],
                                ps.rearrange("p (h e) -> p h e", e=DK))

        # ---------------- phases B + C ----------------
        with tc.tile_pool(name="phB", bufs=1) as bpool, \
             tc.tile_pool(name="pt", bufs=3) as ptpool, \
             tc.tile_pool(name="bcs", bufs=2) as bcpool, \
             tc.tile_pool(name="rc", bufs=2) as rcpool, \
             tc.tile_pool(name="outst", bufs=3) as opool, \
             tc.tile_pool(name="psT", bufs=2, space=bass.MemorySpace.PSUM) as psT, \
             tc.tile_pool(name="psAV", bufs=2, space=bass.MemorySpace.PSUM) as psAV, \
             tc.tile_pool(name="psBC", bufs=2, space=bass.MemorySpace.PSUM) as psBC, \
             tc.tile_pool(name="psO", bufs=2, space=bass.MemorySpace.PSUM) as psO:
            AT = bpool.tile([128, 4, S], F32R, tag="AT")
            wo_sb = bpool.tile([128, 4, D], F32R, tag="wo")
            nc.sync.dma_start(out=wo_sb, in_=woT.rearrange("(c p) n -> p c n", p=128))

            norm_pending = None

            def norm_flush():
                av_, hf_, ob_, qsl_ = norm_pending
                rc = rcpool.tile([1, 512], F32, tag="rc")
                nc.vector.reciprocal(rc, av_[DK:DK + 1, :])
                rcr = rcpool.tile([1, 512], F32R, tag="rcr")
                nc.vector.tensor_copy(rcr, rc)
                bc_ps = psBC.tile([128, 512], F32, tag="bc")
                nc.tensor.matmul(bc_ps, ones[:, 0:128], rcr,
                                 start=True, stop=True)
                bcs = bcpool.tile([DK, 512], F32, tag="bcs")
                nc.scalar.copy(bcs, bc_ps[0:DK, :])
                nc.vector.tensor_mul(AT[hf_:hf_ + DK, ob_, qsl_],
                                     av_[0:DK, :], bcs)

            for qt in range(NQT):
                qsl = slice(512 * qt, 512 * (qt + 1))
                if qt == 0:
                    nc.vector.tensor_copy(qpad0[0:DK, :], QT[0:DK, 0, qsl])
                for h in range(HPC):
                    ob, hf = h // 2, (h % 2) * DK
                    qpad = qpad0 if h % 2 == 0 else qpad1
                    vsl = slice(VSTRIDE * h, VSTRIDE * h + 128)
                    active = [kb for kb in range(NB)
                              if any(cls[kb][4 * qt + j] for j in range(4))]
                    if not active:
                        nc.vector.tensor_copy(AT[hf:hf + DK, ob, qsl],
                                              zeros[0:DK, :])
                        continue
                    av = psAV.tile([128, 512], F32, tag="av")
                    pending = None  # (kb, ptile, c0) awaiting its AV matmul

                    def flush(stop):
                        kb_, pt_, c0_ = pending
                        nc.tensor.matmul(
                            av[:, 128 * c0_:], VA[:, kb_, vsl],
                            pt_[:, 128 * c0_:],
                            start=(kb_ == active[0]), stop=stop)

                    for kb in active:
                        sub = [cls[kb][4 * qt + j] for j in range(4)]
                        if causal:
                            c0 = kb - 4 * qt if kb >= 4 * qt else 0
                        else:
                            c0 = 0
                        if kb == active[0]:
                            c0 = 0  # first AV matmul must cover all columns
                        pt_ps = psT.tile([128, 512], F32, tag="pt")
                        nc.tensor.matmul(pt_ps[:, 128 * c0:],
                                         KT[:, ob, 128 * kb:128 * (kb + 1)],
                                         qpad[:, 128 * c0:],
                                         start=True, stop=True)
                        ptile = ptpool.tile([128, 512], F32R, tag="ptile")
                        nc.scalar.activation(
                            ptile[:, 128 * c0:], pt_ps[:, 128 * c0:],
                            mybir.ActivationFunctionType.Exp, scale=0.125)
                        for j in range(c0, 4):
                            jsl = slice(128 * j, 128 * (j + 1))
                            if sub[j] == 0:
                                nc.vector.tensor_copy(ptile[:, jsl],
                                                      zeros[:, 0:128])
                            elif sub[j] >= 2:
                                nc.vector.tensor_mul(
                                    ptile[:, jsl], ptile[:, jsl],
                                    mask_sb[:, sub[j] - 2, :])
                        if pending is not None:
                            flush(stop=False)
                        pending = (kb, ptile, c0)
                    flush(stop=True)

                    # prefetch the next head's qpad staging copy ahead of
                    # the deferred normalize chain so the next scores matmul
                    # never waits on DVE's reciprocal backlog
                    if h + 1 < HPC:
                        h2 = h + 1
                        nc.vector.tensor_copy(
                            (qpad0 if h2 % 2 == 0 else qpad1)[
                                (h2 % 2) * DK:(h2 % 2) * DK + DK, :],
                            QT[(h2 % 2) * DK:(h2 % 2) * DK + DK, h2 // 2, qsl])
                    elif qt + 1 < NQT:
                        nqsl = slice(512 * (qt + 1), 512 * (qt + 2))
                        nc.vector.tensor_copy(qpad0[0:DK, :],
                                              QT[0:DK, 0, nqsl])
                    # the ~3.3us DVE reciprocal and the whole normalize
                    # chain run one head behind, so neither PE nor the next
                    # head's DVE staging waits on them
                    if norm_pending is not None:
                        norm_flush()
                    norm_pending = (av, hf, ob, qsl)
                if norm_pending is not None:
                    norm_flush()
                    norm_pending = None

                # out-projection for this q-tile
                for s2 in range(4):
                    sb = 4 * qt + s2
                    for ct in range(2):
                        csl = slice(512 * ct, 512 * (ct + 1))
                        po = psO.tile([128, 512], F32, tag="po")
                        for hb in range(4):
                            nc.tensor.matmul(
                                po, AT[:, hb, 128 * sb:128 * (sb + 1)],
                                wo_sb[:, hb, csl],
                                start=(hb == 0), stop=(hb == 3))
                        osb = opool.tile([128, 512], F32, tag="osb")
                        nc.scalar.copy(osb, po)
                        nc.sync.dma_start(
                            out=y.rearrange("(b p) c -> b p c", p=128)[sb][:, csl],
                            in_=osb)

    nc.compile()
    return nc


def _classify_mask(mask2d):
    """Return (cls 16x16 list, mask_tiles list, causal flag) for the T
    orientation: cls[kb][qb] over 128x128 blocks of mask2d[q, k]."""
    m = (np.asarray(mask2d) != 0)
    blocks = m.reshape(NB, 128, NB, 128)  # [qb, ql, kb, kl]
    cls = [[0] * NB for _ in range(NB)]
    tiles = []
    keys = {}
    for kb in range(NB):
        for qb in range(NB):
            blk = blocks[qb, :, kb, :]  # [ql, kl]
            s = int(blk.sum())
            if s == 0:
                cls[kb][qb] = 0
            elif s == 128 * 128:
                cls[kb][qb] = 1
            else:
                t = np.ascontiguousarray(blk.T).astype(np.float32)  # [kl, ql]
                key = t.tobytes()
                if key not in keys:
                    keys[key] = len(tiles)
                    tiles.append(t)
                cls[kb][qb] = 2 + keys[key]
    causal = bool(np.array_equal(m, np.tril(np.ones((S, S), bool))))
    return cls, tiles, causal


def _vinit_plane():
    v = np.zeros((128, NB, HPC * VSTRIDE + 64), np.float32)
    for h in range(HPC):
        v[:, :, VSTRIDE * h + DK] = 1.0
    return v


_PROGRAM_CACHE = {}


def _get_program(mask2d):
    cls, tiles, causal = _classify_mask(mask2d)
    key = (tuple(tuple(r) for r in cls),
           tuple(t.tobytes() for t in tiles), causal)
    if key not in _PROGRAM_CACHE:
        _PROGRAM_CACHE[key] = (build_program(cls, tiles, causal), tiles, causal)
    return _PROGRAM_CACHE[key]


def run(inputs, trace=False):
    query = np.asarray(inputs["query"], np.float32)
    key_ = np.asarray(inputs["key"], np.float32)
    value = np.asarray(inputs["value"], np.float32)
    mask = np.asarray(inputs["mask"])
    Wq = np.asarray(inputs["Wq"], np.float32)
    bq = np.asarray(inputs["bq"], np.float32)
    Wk = np.asarray(inputs["Wk"], np.float32)
    bk = np.asarray(inputs["bk"], np.float32)
    Wv = np.asarray(inputs["Wv"], np.float32)
    bv = np.asarray(inputs["bv"], np.float32)
    Wo = np.asarray(inputs["Wo"], np.float32)
    bo = np.asarray(inputs["bo"], np.float32)

    nc, tiles, causal_flag = _get_program(mask[0, 0])

    in_maps = []
    for core in range(N_CORES):
        b, hg = core // 2, core % 2
        osl = slice(OC * hg, OC * (hg + 1))
        im = {
            "xqT": np.ascontiguousarray(query[b].T),
            "xkT": np.ascontiguousarray(key_[b].T),
            "xvT": np.ascontiguousarray(value[b].T),
            "wqT": np.ascontiguousarray(Wq.T[:, osl]),
            "wkT": np.ascontiguousarray(Wk.T[:, osl]),
            "wvT": np.ascontiguousarray(Wv.T[:, osl]),
            "bq": bq[osl].copy(),
            "bk": bk[osl].copy(),
            "bv": bv[osl].copy(),
            "woT": np.ascontiguousarray(Wo.T[osl, :]),
            "ones_row": np.ones(512, np.float32),
            "vinit": _vinit_plane(),
            "zeros": np.zeros((128, 512), np.float32),
        }
        if tiles:
            im["masks"] = np.stack(tiles)
        in_maps.append(im)

    res = run_bass_kernel_spmd(nc, in_maps, list(range(N_CORES)), trace=trace)
    out = np.empty((B, S, D), np.float32)
    for b in range(B):
        out[b] = res.results[2 * b]["y"] + res.results[2 * b + 1]["y"]
    out += bo
    return out, res


def kernel(**inputs):
    out, _ = run(inputs, trace=False)
    return out


# revision 14
# speedup vs baseline: 1.3042x; 1.0711x over previous
"""Multi-head attention (B=4, S=2048, D=1024, H=16, causal) on 8 trn2 NeuronCores.

Sharding: core i handles batch b = i//2 and head-group hg = i%2 (8 heads each).
Data-parallel over B, tensor-parallel over heads; the out-projection partial
sums of the two head-groups of a batch are reduced on the host. No collectives.

Per-core dataflow (all matmuls in float32r, typed end-to-end — the BIR
verifier requires fp32r matmul operands to be produced as fp32r):
  phase A: QT[o,s], KT[o,s] (head-transposed) and V[s,o] (natural, augmented
           with a ones-column per head) via projections from host-transposed
           activations; biases folded in with K=1 augmented matmuls.
  phase B: per (q-tile, head): scores T[k,q] = KT_blk @ QT_tile on PE,
           exp on ScalarE (no max subtraction; causal scores are O(+-6)),
           causal masking via block skipping + one triangular mask tile,
           P.T @ [V|1] accumulation gives attention output (transposed) and
           softmax denominators in one PSUM tile; normalize via PE-broadcast
           of reciprocal denominators.
  phase C: out-projection with A.T blocks as stationary operands; per-core
           partial y (bo added on host).
"""

import numpy as np
from contextlib import ExitStack

import concourse.bass as bass
import concourse.tile as tile
from concourse import bacc, mybir
from concourse.bass_utils import run_bass_kernel_spmd

F32 = mybir.dt.float32
F32R = mybir.dt.float32r
F16 = mybir.dt.float16

B, S, D, H, DK = 4, 2048, 1024, 16, 64
HPC = 8          # heads per core
OC = HPC * DK    # 512 out-cols per core
NB = S // 128    # 16 seq blocks of 128
NQT = S // 512   # 4 q-tiles of 512
N_CORES = 8
VSTRIDE = DK + 1  # V cols per head incl the ones column


def build_program(cls, mask_tiles, causal):
    """cls[kb][qb] for the 16x16 grid of 128x128 blocks (T orientation:
    kb = key block, qb = query block): 0 = fully masked, 1 = fully valid,
    >=2 -> mixed, multiply by mask_tiles[cls-2] after exp."""
    n_mask = len(mask_tiles)
    nc = bacc.Bacc("TRN2", target_bir_lowering=False, debug=False,
                   num_devices=N_CORES, enable_asserts=False)

    xqT = nc.dram_tensor("xqT", [D, S], F32R, kind="ExternalInput").ap()
    xkT = nc.dram_tensor("xkT", [D, S], F32R, kind="ExternalInput").ap()
    xvT = nc.dram_tensor("xvT", [D, S], F32R, kind="ExternalInput").ap()
    wqT = nc.dram_tensor("wqT", [D, OC], F32R, kind="ExternalInput").ap()
    wkT = nc.dram_tensor("wkT", [D, OC], F32R, kind="ExternalInput").ap()
    wvT = nc.dram_tensor("wvT", [D, OC], F32R, kind="ExternalInput").ap()
    bq = nc.dram_tensor("bq", [OC], F32, kind="ExternalInput").ap()
    bk = nc.dram_tensor("bk", [OC], F32, kind="ExternalInput").ap()
    bv = nc.dram_tensor("bv", [OC], F32R, kind="ExternalInput").ap()
    woT = nc.dram_tensor("woT", [OC, D], F32R, kind="ExternalInput").ap()
    ones_row_d = nc.dram_tensor("ones_row", [512], F32R,
                                kind="ExternalInput").ap()
    vinit_d = nc.dram_tensor("vinit", [128, NB, HPC * VSTRIDE + 64], F16,
                             kind="ExternalInput").ap()
    zeros_d = nc.dram_tensor("zeros", [128, 512], F32R,
                             kind="ExternalInput").ap()
    masks = None
    if n_mask:
        masks = nc.dram_tensor("masks", [n_mask, 128, 128], F16,
                               kind="ExternalInput").ap()
    y = nc.dram_tensor("y", [S, D], F32, kind="ExternalOutput").ap()

    with tile.TileContext(nc) as tc, ExitStack() as ctx:
        persist = ctx.enter_context(tc.tile_pool(name="persist", bufs=1))
        QT = persist.tile([128, 4, S], F32R, tag="QT")
        KT = persist.tile([128, 4, S], F32R, tag="KT")
        # VSTRIDE*HPC data cols + 64 zero pad cols so the AV stationary can
        # always be a full [128, 128] window (M=128 is the fast LDW path)
        VA = persist.tile([128, NB, HPC * VSTRIDE + 64], F16, tag="VA")
        ones = persist.tile([1, 512], F32R, tag="ones")
        nc.sync.dma_start(out=ones, in_=ones_row_d[None, :])
        zeros = None
        if not causal:
            zeros = persist.tile([128, 512], F32R, tag="zeros")
            nc.sync.dma_start(out=zeros, in_=zeros_d)
        # Q staging tiles, zero-padded so the scores matmul can contract over
        # the full 128 partitions (two-head KT block x one-head padded Q).
        # Even h uses qpad0 (head rows 0:64), odd h uses qpad1 (rows 64:128);
        # the other half of each stays zero forever.
        qpad0 = persist.tile([128, 512], F32R, tag="qpad0")
        qpad1 = persist.tile([128, 512], F32R, tag="qpad1")
        nc.sync.dma_start(out=qpad0, in_=zeros_d)
        nc.sync.dma_start(out=qpad1, in_=zeros_d)
        bq_pp = persist.tile([128, 4], F32, tag="bqp")
        bk_pp = persist.tile([128, 4], F32, tag="bkp")
        bv_sb = persist.tile([1, OC], F32R, tag="bv")
        nc.sync.dma_start(out=bq_pp, in_=bq.rearrange("(ob p) -> p ob", p=128))
        nc.sync.dma_start(out=bk_pp, in_=bk.rearrange("(ob p) -> p ob", p=128))
        nc.sync.dma_start(out=bv_sb, in_=bv[None, :])
        mask_sb = None
        if n_mask:
            mask_sb = persist.tile([128, n_mask, 128], F16, tag="mask")
            nc.sync.dma_start(out=mask_sb, in_=masks.rearrange("m p j -> p m j"))
        # ones columns + zero pad, one efficient full-plane DMA (the V data
        # region is overwritten by the projection copies afterwards)
        nc.sync.dma_start(out=VA, in_=vinit_d)

        # ---------------- phase A: projections ----------------
        with tc.tile_pool(name="phA_w", bufs=1) as wpool, \
             tc.tile_pool(name="phA_x", bufs=2) as xpool, \
             tc.tile_pool(name="phA_ps", bufs=4, space=bass.MemorySpace.PSUM) as pps:
            wq_sb = wpool.tile([128, 8, OC], F32R, tag="wq")
            wk_sb = wpool.tile([128, 8, OC], F32R, tag="wk")
            wv_sb = wpool.tile([128, 8, OC], F32R, tag="wv")
            for c in range(8):
                nc.sync.dma_start(
                    out=wq_sb[:, c, :],
                    in_=wqT.rearrange("(c p) o -> p c o", p=128)[:, c, :])

            for which in ("q", "k", "v"):
                if which == "k":
                    for c in range(8):
                        nc.sync.dma_start(
                            out=wk_sb[:, c, :],
                            in_=wkT.rearrange("(c p) o -> p c o", p=128)[:, c, :])
                if which == "v":
                    for c in range(8):
                        nc.sync.dma_start(
                            out=wv_sb[:, c, :],
                            in_=wvT.rearrange("(c p) o -> p c o", p=128)[:, c, :])
                for st in range(NQT):
                    ssl = slice(512 * st, 512 * (st + 1))
                    xsrc = {"q": xqT, "k": xkT, "v": xvT}[which]
                    xs = xpool.tile([128, 8, 512], F32R, tag="xstage")
                    for c in range(8):
                        nc.sync.dma_start(
                            out=xs[:, c, :],
                            in_=xsrc.rearrange("(c p) s -> p c s",
                                               p=128)[:, c, ssl])
                    if which in ("q", "k"):
                        wsb = wq_sb if which == "q" else wk_sb
                        bpp = bq_pp if which == "q" else bk_pp
                        dst = QT if which == "q" else KT
                        for ob in range(4):
                            osl = slice(128 * ob, 128 * (ob + 1))
                            ps = pps.tile([128, 512], F32, tag="ps")
                            for c in range(8):
                                nc.tensor.matmul(ps, wsb[:, c, osl], xs[:, c, :],
                                                 start=(c == 0),
                                                 stop=(c == 7))
                            nc.vector.tensor_scalar_add(
                                dst[:, ob, ssl], ps, bpp[:, ob:ob + 1])
                    else:
                        for s2 in range(4):
                            sb = 4 * st + s2
                            ps = pps.tile([128, 512], F32, tag="ps")
                            for c in range(8):
                                nc.tensor.matmul(
                                    ps, xs[:, c, 128 * s2:128 * (s2 + 1)],
                                    wv_sb[:, c, :],
                                    start=(c == 0), stop=False)
                            nc.tensor.matmul(ps, ones[:, 0:128], bv_sb,
                                             start=False, stop=True)
                            nc.vector.tensor_copy(
                                VA[:, sb, 0:HPC * VSTRIDE].rearrange(
                                    "p (h e) -> p h e",
                                    e=VSTRIDE)[:, :, 0:DK],
                                ps.rearrange("p (h e) -> p h e", e=DK))

        # ---------------- phases B + C ----------------
        with tc.tile_pool(name="phB", bufs=1) as bpool, \
             tc.tile_pool(name="pt", bufs=3) as ptpool, \
             tc.tile_pool(name="bcs", bufs=2) as bcpool, \
             tc.tile_pool(name="rc", bufs=2) as rcpool, \
             tc.tile_pool(name="outst", bufs=3) as opool, \
             tc.tile_pool(name="psT", bufs=2, space=bass.MemorySpace.PSUM) as psT, \
             tc.tile_pool(name="psAV", bufs=3, space=bass.MemorySpace.PSUM) as psAV, \
             tc.tile_pool(name="psBC", bufs=1, space=bass.MemorySpace.PSUM) as psBC, \
             tc.tile_pool(name="psO", bufs=2, space=bass.MemorySpace.PSUM) as psO:
            AT = bpool.tile([128, 4, S], F32R, tag="AT")
            wo_sb = bpool.tile([128, 4, D], F32R, tag="wo")
            nc.sync.dma_start(out=wo_sb, in_=woT.rearrange("(c p) n -> p c n", p=128))

            norm_pending = []

            def norm_flush():
                av_, hf_, ob_, qsl_ = norm_pending.pop(0)
                rc = rcpool.tile([1, 512], F32, tag="rc")
                nc.vector.reciprocal(rc, av_[DK:DK + 1, :])
                rcr = rcpool.tile([1, 512], F32R, tag="rcr")
                nc.vector.tensor_copy(rcr, rc)
                bc_ps = psBC.tile([128, 512], F32, tag="bc")
                nc.tensor.matmul(bc_ps, ones[:, 0:128], rcr,
                                 start=True, stop=True)
                bcs = bcpool.tile([DK, 512], F32, tag="bcs")
                nc.scalar.copy(bcs, bc_ps[0:DK, :])
                nc.vector.tensor_mul(AT[hf_:hf_ + DK, ob_, qsl_],
                                     av_[0:DK, :], bcs)

            for qt in range(NQT):
                qsl = slice(512 * qt, 512 * (qt + 1))
                if qt == 0:
                    nc.vector.tensor_copy(qpad0[0:DK, :], QT[0:DK, 0, qsl])
                for h in range(HPC):
                    ob, hf = h // 2, (h % 2) * DK
                    qpad = qpad0 if h % 2 == 0 else qpad1
                    vsl = slice(VSTRIDE * h, VSTRIDE * h + 128)
                    active = [kb for kb in range(NB)
                              if any(cls[kb][4 * qt + j] for j in range(4))]
                    if not active:
                        nc.vector.tensor_copy(AT[hf:hf + DK, ob, qsl],
                                              zeros[0:DK, :])
                        continue
                    av = psAV.tile([128, 512], F32, tag="av")
                    pending = None  # (kb, ptile, c0) awaiting its AV matmul

                    def flush(stop):
                        kb_, pt_, c0_ = pending
                        nc.tensor.matmul(
                            av[:, 128 * c0_:], VA[:, kb_, vsl],
                            pt_[:, 128 * c0_:],
                            start=(kb_ == active[0]), stop=stop)

                    for kb in active:
                        sub = [cls[kb][4 * qt + j] for j in range(4)]
                        if causal:
                            c0 = kb - 4 * qt if kb >= 4 * qt else 0
                        else:
                            c0 = 0
                        if kb == active[0]:
                            c0 = 0  # first AV matmul must cover all columns
                        pt_ps = psT.tile([128, 512], F32, tag="pt")
                        nc.tensor.matmul(pt_ps[:, 128 * c0:],
                                         KT[:, ob, 128 * kb:128 * (kb + 1)],
                                         qpad[:, 128 * c0:],
                                         start=True, stop=True)
                        ptile = ptpool.tile([128, 512], F16, tag="ptile")
                        nc.scalar.activation(
                            ptile[:, 128 * c0:], pt_ps[:, 128 * c0:],
                            mybir.ActivationFunctionType.Exp, scale=0.125)
                        for j in range(c0, 4):
                            jsl = slice(128 * j, 128 * (j + 1))
                            if sub[j] == 0:
                                nc.vector.tensor_copy(ptile[:, jsl],
                                                      zeros[:, 0:128])
                            elif sub[j] >= 2:
                                nc.vector.tensor_mul(
                                    ptile[:, jsl], ptile[:, jsl],
                                    mask_sb[:, sub[j] - 2, :])
                        if pending is not None:
                            flush(stop=False)
                        pending = (kb, ptile, c0)
                    flush(stop=True)

                    # prefetch the next head's qpad staging copy ahead of
                    # the deferred normalize chain so the next scores matmul
                    # never waits on DVE's reciprocal backlog
                    if h + 1 < HPC:
                        h2 = h + 1
                        nc.vector.tensor_copy(
                            (qpad0 if h2 % 2 == 0 else qpad1)[
                                (h2 % 2) * DK:(h2 % 2) * DK + DK, :],
                            QT[(h2 % 2) * DK:(h2 % 2) * DK + DK, h2 // 2, qsl])
                    elif qt + 1 < NQT:
                        nqsl = slice(512 * (qt + 1), 512 * (qt + 2))
                        nc.vector.tensor_copy(qpad0[0:DK, :],
                                              QT[0:DK, 0, nqsl])
                    # the ~3.3us DVE reciprocal and the whole normalize
                    # chain run two heads behind, so neither PE nor the next
                    # head's DVE staging waits on them
                    norm_pending.append((av, hf, ob, qsl))
                    if len(norm_pending) > 2:
                        norm_flush()
                while norm_pending:
                    norm_flush()

                # out-projection for this q-tile
                for s2 in range(4):
                    sb = 4 * qt + s2
                    for ct in range(2):
                        csl = slice(512 * ct, 512 * (ct + 1))
                        po = psO.tile([128, 512], F32, tag="po")
                        for hb in range(4):
                            nc.tensor.matmul(
                                po, AT[:, hb, 128 * sb:128 * (sb + 1)],
                                wo_sb[:, hb, csl],
                                start=(hb == 0), stop=(hb == 3))
                        osb = opool.tile([128, 512], F32, tag="osb")
                        nc.scalar.copy(osb, po)
                        nc.sync.dma_start(
                            out=y.rearrange("(b p) c -> b p c", p=128)[sb][:, csl],
                            in_=osb)

    nc.compile()
    return nc


def _classify_mask(mask2d):
    """Return (cls 16x16 list, mask_tiles list, causal flag) for the T
    orientation: cls[kb][qb] over 128x128 blocks of mask2d[q, k]."""
    m = (np.asarray(mask2d) != 0)
    blocks = m.reshape(NB, 128, NB, 128)  # [qb, ql, kb, kl]
    cls = [[0] * NB for _ in range(NB)]
    tiles = []
    keys = {}
    for kb in range(NB):
        for qb in range(NB):
            blk = blocks[qb, :, kb, :]  # [ql, kl]
            s = int(blk.sum())
            if s == 0:
                cls[kb][qb] = 0
            elif s == 128 * 128:
                cls[kb][qb] = 1
            else:
                t = np.ascontiguousarray(blk.T).astype(np.float32)  # [kl, ql]
                key = t.tobytes()
                if key not in keys:
                    keys[key] = len(tiles)
                    tiles.append(t)
                cls[kb][qb] = 2 + keys[key]
    causal = bool(np.array_equal(m, np.tril(np.ones((S, S), bool))))
    return cls, tiles, causal


def _vinit_plane():
    v = np.zeros((128, NB, HPC * VSTRIDE + 64), np.float16)
    for h in range(HPC):
        v[:, :, VSTRIDE * h + DK] = 1.0
    return v


_PROGRAM_CACHE = {}


def _get_program(mask2d):
    cls, tiles, causal = _classify_mask(mask2d)
    key = (tuple(tuple(r) for r in cls),
           tuple(t.tobytes() for t in tiles), causal)
    if key not in _PROGRAM_CACHE:
        _PROGRAM_CACHE[key] = (build_program(cls, tiles, causal), tiles, causal)
    return _PROGRAM_CACHE[key]


def run(inputs, trace=False):
    query = np.asarray(inputs["query"], np.float32)
    key_ = np.asarray(inputs["key"], np.float32)
    value = np.asarray(inputs["value"], np.float32)
    mask = np.asarray(inputs["mask"])
    Wq = np.asarray(inputs["Wq"], np.float32)
    bq = np.asarray(inputs["bq"], np.float32)
    Wk = np.asarray(inputs["Wk"], np.float32)
    bk = np.asarray(inputs["bk"], np.float32)
    Wv = np.asarray(inputs["Wv"], np.float32)
    bv = np.asarray(inputs["bv"], np.float32)
    Wo = np.asarray(inputs["Wo"], np.float32)
    bo = np.asarray(inputs["bo"], np.float32)

    nc, tiles, causal_flag = _get_program(mask[0, 0])

    in_maps = []
    for core in range(N_CORES):
        b, hg = core // 2, core % 2
        osl = slice(OC * hg, OC * (hg + 1))
        im = {
            "xqT": np.ascontiguousarray(query[b].T),
            "xkT": np.ascontiguousarray(key_[b].T),
            "xvT": np.ascontiguousarray(value[b].T),
            "wqT": np.ascontiguousarray(Wq.T[:, osl]),
            "wkT": np.ascontiguousarray(Wk.T[:, osl]),
            "wvT": np.ascontiguousarray(Wv.T[:, osl]),
            "bq": bq[osl].copy(),
            "bk": bk[osl].copy(),
            "bv": bv[osl].copy(),
            "woT": np.ascontiguousarray(Wo.T[osl, :]),
            "ones_row": np.ones(512, np.float32),
            "vinit": _vinit_plane(),
            "zeros": np.zeros((128, 512), np.float32),
        }
        if tiles:
            im["masks"] = np.stack(tiles).astype(np.float16)
        in_maps.append(im)

    res = run_bass_kernel_spmd(nc, in_maps, list(range(N_CORES)), trace=trace)
    out = np.empty((B, S, D), np.float32)
    for b in range(B):
        out[b] = res.results[2 * b]["y"] + res.results[2 * b + 1]["y"]
    out += bo
    return out, res


def kernel(**inputs):
    out, _ = run(inputs, trace=False)
    return out
